# revision 1
# baseline (speedup 1.0000x reference)
"""EnergyNet Trainium2 kernel v2 (SPMD over 8 NeuronCores).

Layout: partitions = j (columns of the reference's NxN pairwise maps),
free dim = i (rows). Each core owns 256 j's (2 tiles of 128). All
multiplicative j-factors are per-partition scalars; additive i-terms ride
in PE matmuls / DMA-broadcast tiles; multiplicative i-factors (qs_i, c_i,
sfb_i) are applied on the host to the reduced rows.

Per-core i-axis is rotated by -256*core so the (i==j) diagonal sits at a
core-independent column. An identity-matmul "poke" adds 1e6 to the D^2 of
the diagonal and of all near pairs (D < 0.5), which the fp32 Gram
decomposition cannot resolve; their exact contributions are added on the
host (their device-side residuals are ~1e-3 and exactly mask-cancelled).
"""
import os
import numpy as np
import ml_dtypes

import concourse.bass as bass
import concourse.mybir as mybir
import bass_rust as _bass_rust
from concourse.bass_utils import run_bass_kernel_spmd
from concourse.tile import TileContext

N = 2048
C = 8
CONV = 332.07156
NCORES = 8
P = 128
JT = 2
JPC = P * JT
LN5 = float(np.log(5.0))
DIAG_BIG = 1.0e6
NEAR_TH2 = 0.25

AF = mybir.ActivationFunctionType
ALU = mybir.AluOpType
F32 = mybir.dt.float32
BF16 = mybir.dt.bfloat16


# --------------------------------------------------------------- patches
def _patched_drain_and_barrier(self, tick_clock, wait_clock):
    gc = tick_clock.global_clock
    try:
        n_procs = len(gc)
    except TypeError:
        n_procs = 27
    ticks = [gc[p] for p in range(n_procs)]
    for p in [p for p in range(n_procs) if ticks[p] > 0] or [0]:
        d = self.nc.sync.drain()
        sub = [ticks[q] if q == p else 0 for q in range(n_procs)]
        wait_clock.add_sem_waits(
            d.ins, _bass_rust.ScopedClock({None: _bass_rust.VectorClock(sub)})
        )
    self.nc.all_engine_barrier()
    assert self.sems is not None
    popped = self.nc._tile_sem_poison_stack.pop()
    assert popped is self._sem_poison
    self.nc.clear_and_free_semaphores(list(self.sems.allocated().values()))
    self.nc.all_engine_barrier()


TileContext._drain_and_barrier = _patched_drain_and_barrier

_NOPC = [0]


def _split_excess_waits(nc):
    """This walrus build rejects instructions carrying more than one sem
    wait. Hoist excess waits onto same-engine NoOps inserted just before
    the offending instruction (the engine sequencer executes them in
    order, so the waits still gate it)."""
    for blk in nc.m.functions[0].blocks:
        insts = blk.instructions
        out = []
        changed = False
        for inst in insts:
            si = inst.sync_info
            waits = list(si.on_wait) if si is not None else []
            if len(waits) > 1:
                keep_idx = len(waits) - 1
                if type(inst).__name__ == "InstDMACopy":
                    for k, w in enumerate(waits):
                        if str(getattr(w, "ant_name", "")).startswith(
                                ("DMAHW", "DMASW")):
                            keep_idx = k
                            break
                rest = [w for k, w in enumerate(waits) if k != keep_idx]
                for w in rest:
                    _NOPC[0] += 1
                    nop = mybir.InstNoOp(name=f"WH-{_NOPC[0]}", ins=[], outs=[])
                    nop.engine = inst.engine
                    nop.sync_info = mybir.SyncInfo(on_wait=[w], on_update=[])
                    out.append(nop)
                inst.sync_info = mybir.SyncInfo(on_wait=[waits[keep_idx]],
                                                on_update=list(si.on_update))
                changed = True
            out.append(inst)
        if changed:
            blk.instructions = out


def _bcast_src(dram_ap, n_free):
    """Stride-0 partition AP: read one DRAM row into all 128 partitions."""
    return bass.AP(tensor=dram_ap.tensor, offset=0,
                   ap=_bass_rust.VecI64Pair([[0, P], [1, n_free]]))


_CACHE = {}


def _build():
    if "nc" in _CACHE:
        return _CACHE["nc"]
    nc = bass.Bass()
    geo = nc.declare_dram_parameter("geo", [4, N + JT * P], F32, isOutput=False)
    brrow = nc.declare_dram_parameter("brrow", [1, N], F32, isOutput=False)
    bdrow = nc.declare_dram_parameter("bdrow", [1, N], F32, isOutput=False)
    scal = nc.declare_dram_parameter("scal", [P, 8 * JT], F32, isOutput=False)
    wtsb = nc.declare_dram_parameter("wtsb", [P, 8 * JT], BF16, isOutput=False)
    pkid = nc.declare_dram_parameter("pkid", [P, P], BF16, isOutput=False)
    pk = nc.declare_dram_parameter("pk", [P, JT * N], BF16, isOutput=False)
    rows_out = nc.declare_dram_parameter("rows", [66, N], F32, isOutput=True)

    with TileContext(nc) as tc:
        with tc.tile_pool(name="const", bufs=1) as cpool, \
             tc.tile_pool(name="work", bufs=1) as wpool, \
             tc.tile_pool(name="pbig", bufs=1, space="PSUM") as pbig, \
             tc.tile_pool(name="prows", bufs=1, space="PSUM") as prows:

            t_geo = cpool.tile([4, N + JT * P], F32, name="t_geo")
            t_scal = cpool.tile([P, 8 * JT], F32, name="t_scal")
            t_wtsb = cpool.tile([P, 8 * JT], BF16, name="t_wtsb")
            t_pkid = cpool.tile([P, P], BF16, name="t_pkid")
            t_pk = cpool.tile([P, JT * N], BF16, name="t_pk")
            t_Bbr = wpool.tile([P, N], F32, name="t_Bbr", tag="bbr")
            t_Bbd = wpool.tile([P, N], F32, name="t_Bbd", tag="bbd")
            nc.sync.dma_start(t_geo[:], geo[:])
            nc.sync.dma_start(t_scal[:], scal[:])
            nc.sync.dma_start(t_wtsb[:], wtsb[:])
            nc.sync.dma_start(t_pkid[:], pkid[:])
            nc.sync.dma_start(t_pk[:], pk[:])
            nc.sync.dma_start(t_Bbr[:], _bcast_src(brrow[:], N))
            nc.sync.dma_start(t_Bbd[:], _bcast_src(bdrow[:], N))

            ps_rows = prows.tile([66, N], F32, name="ps_rows")

            def sc(t, k):
                return t_scal[:, 8 * t + k:8 * t + k + 1]

            # ---- stage B: D2 maps (+pokes) and D = sqrt (sqrt set)
            from concourse.tile import add_dep_helper
            Ds, sqs = [], []
            last_D = None
            for t in range(JT):
                ps = pbig.tile([P, N], F32, name=f"ps_d2_{t}", tag="psbig")
                lhsT = t_geo[0:4, N + P * t:N + P * (t + 1)]
                for ch in range(4):
                    sl = slice(ch * 512, (ch + 1) * 512)
                    nc.tensor.matmul(ps[:, sl], lhsT, t_geo[0:4, sl],
                                     start=True, stop=False)
                    nc.tensor.matmul(ps[:, sl], t_pkid[:],
                                     t_pk[:, t * N + ch * 512:
                                          t * N + (ch + 1) * 512],
                                     start=False, stop=True)
                Dt = wpool.tile([P, N], F32, name=f"D_{t}")
                nc.scalar.activation(Dt[:], ps[:], AF.Sqrt, bias=sc(t, 0))
                sqt = wpool.tile([P, N], F32, name=f"sq_{t}")
                nc.scalar.activation(sqt[:], Dt[:], AF.Square)
                Ds.append(Dt); sqs.append(sqt)

            # ---- stage A: sigmoids -> s, w3 (sigmoid set, ready at start;
            # overlaps the PE D2 matmuls)
            ss, w3s = [], []
            last_sig = None
            for t in range(JT):
                sig = wpool.tile([P, N], F32, name=f"sig_{t}", tag="sig")
                nc.scalar.activation(sig[:], t_Bbr[:], AF.Sigmoid, bias=sc(t, 1))
                s_t = wpool.tile([P, N], F32, name=f"s_{t}")
                nc.gpsimd.tensor_scalar(s_t[:], sig[:], sc(t, 3), sc(t, 4),
                                        ALU.mult, ALU.add)
                sig2 = wpool.tile([P, N], F32, name=f"sig2_{t}", tag="sig2")
                last_sig = nc.scalar.activation(sig2[:], t_Bbd[:], AF.Sigmoid,
                                                bias=sc(t, 2))
                w3 = wpool.tile([P, N], BF16, name=f"w3_{t}")
                nc.gpsimd.tensor_scalar(w3[:], sig2[:], sc(t, 5), sc(t, 6),
                                        ALU.mult, ALU.add)
                ss.append(s_t); w3s.append(w3)

            # ---- stage 3: per-tile chains (exp set)
            for t in range(JT):
                Dt, sqt, s_t, w3 = Ds[t], sqs[t], ss[t], w3s[t]
                first, last = (t == 0), (t == JT - 1)

                Dm = wpool.tile([P, N], F32, name=f"Dm_{t}")
                nc.vector.tensor_tensor(Dm[:], Dt[:], s_t[:], ALU.subtract)
                q = wpool.tile([P, N], BF16, name=f"q_{t}")
                nc.vector.tensor_tensor(q[:], Dm[:], Dm[:], ALU.mult)
                u = wpool.tile([P, N], BF16, name=f"u_{t}")
                nc.gpsimd.tensor_scalar(u[:], Dm[:], 0.6, -0.09,
                                        ALU.mult, ALU.add)
                nc.vector.tensor_tensor(u[:], u[:], q[:], ALU.subtract)

                invD = wpool.tile([P, N], BF16, name=f"invD_{t}")
                with nc.allow_low_precision(reason="invD rounds to bf16; "
                                            "reduction accumulates fp32 in PSUM"):
                    nc.vector.reciprocal(invD[:], Dt[:])
                invD2 = wpool.tile([P, N], BF16, name=f"invD2_{t}")
                nc.vector.tensor_tensor(invD2[:], invD[:], invD[:], ALU.mult)
                # D3 = D^2 * D (in place over sq)
                nc.vector.tensor_tensor(sqt[:], sqt[:], Dt[:], ALU.mult)

                e3 = wpool.tile([P, N], BF16, name=f"e3_{t}",
                                tag="e3" if t == 0 else "bbr")
                nc.scalar.activation(e3[:], q[:], AF.Exp, scale=-3.0)
                e10 = wpool.tile([P, N], BF16, name=f"e10_{t}",
                                 tag="e10" if t == 0 else "bbd")
                nc.scalar.activation(e10[:], q[:], AF.Exp, scale=-10.0)
                e1 = wpool.tile([P, N], BF16, name=f"e1_{t}")
                nc.scalar.activation(e1[:], u[:], AF.Exp)
                repl5 = wpool.tile([P, N], BF16, name=f"repl5_{t}")
                nc.scalar.activation(repl5[:], sqt[:], AF.Exp, scale=-0.3,
                                     bias=sc(t, 7))

                # S = e1+e3+e10 (into e1); WS = w3*S; vdw = repl5 - WS
                nc.gpsimd.tensor_tensor(e1[:], e1[:], e3[:], ALU.add)
                nc.vector.tensor_tensor(e1[:], e1[:], e10[:], ALU.add)
                WS = wpool.tile([P, N], BF16, name=f"WS_{t}",
                                tag="sig" if t == 0 else "sig2")
                nc.vector.tensor_tensor(WS[:], w3[:], e1[:], ALU.mult)
                nc.vector.tensor_tensor(repl5[:], repl5[:], WS[:], ALU.subtract)

                for ch in range(4):
                    sl = slice(ch * 512, (ch + 1) * 512)
                    nc.tensor.matmul(ps_rows[0:4, sl],
                                     t_wtsb[:, 8 * t:8 * t + 4], invD[:, sl],
                                     start=first, stop=last)
                    nc.tensor.matmul(ps_rows[32:34, sl],
                                     t_wtsb[:, 8 * t + 4:8 * t + 6],
                                     invD2[:, sl], start=first, stop=last)
                    nc.tensor.matmul(ps_rows[64:66, sl],
                                     t_wtsb[:, 8 * t + 6:8 * t + 8],
                                     repl5[:, sl], start=first, stop=last)

            rows_sb = cpool.tile([66, N], F32, name="rows_sb")
            nc.scalar.copy(rows_sb[:], ps_rows[:])
            nc.gpsimd.dma_start(rows_out[:], rows_sb[:])

    _split_excess_waits(nc)
    _CACHE["nc"] = nc
    return nc


# --------------------------------------------------------------- host side
def _host_pre(inputs):
    f32 = np.float32
    X = np.asarray(inputs["X"], f32)
    embs = np.asarray(inputs["embs"], f32)
    qs = np.asarray(inputs["qs"], f32)
    w0 = np.asarray(inputs["w0"], f32)
    s0 = np.asarray(inputs["s0"], f32)
    c = np.asarray(inputs["chainidx"]).astype(f32)
    f = np.asarray(inputs["sf_elec"], f32)[:, 0]
    rf = np.asarray(inputs["radius_factor"], f32)[:, 0]
    df = np.asarray(inputs["depth_factor"], f32)[:, 0]

    Xc = (X.astype(np.float64) - X.astype(np.float64).mean(0)).astype(f32)
    r2 = (Xc.astype(np.float64) ** 2).sum(1).astype(f32)

    sfa = embs @ f[:C]
    sfb = embs @ f[C:2 * C]
    f16 = f[2 * C]
    ar = embs @ rf[:C]
    br = embs @ rf[C:]
    ad = embs @ df[:C]
    bd = embs @ df[C:]
    w0j = np.sqrt(w0 * w0 + 1e-6).astype(f32)
    one_m2c = (1.0 - 2.0 * c).astype(f32)

    # exact pair distances (fp64) to find pairs the fp32 Gram decomposition
    # cannot resolve; they are poked out on device and corrected on host.
    X64 = Xc.astype(np.float64)
    r264 = (X64 ** 2).sum(1)
    D2x = r264[:, None] + r264[None, :] - 2.0 * (X64 @ X64.T)
    np.fill_diagonal(D2x, 1e9)
    near_i, near_j = np.where(D2x < NEAR_TH2)

    pkid_m = (np.eye(P, dtype=np.float32) * DIAG_BIG).astype(ml_dtypes.bfloat16)
    in_maps = []
    for core in range(NCORES):
        rot = lambda a: np.roll(a, -core * JPC, axis=-1)

        geo = np.zeros((4, N + JT * P), f32)
        geo[0, :N] = rot(Xc[:, 0]); geo[1, :N] = rot(Xc[:, 1])
        geo[2, :N] = rot(Xc[:, 2]); geo[3, :N] = rot(r2) + 3e-6
        pk_m = np.zeros((P, JT * N), np.float32)
        scal_m = np.zeros((P, 8 * JT), f32)
        wtsb_m = np.zeros((P, 8 * JT), np.float32)
        for t in range(JT):
            jj = slice(core * JPC + t * P, core * JPC + (t + 1) * P)
            cl = slice(N + t * P, N + (t + 1) * P)
            geo[0, cl] = -2.0 * Xc[jj, 0]
            geo[1, cl] = -2.0 * Xc[jj, 1]
            geo[2, cl] = -2.0 * Xc[jj, 2]
            geo[3, cl] = 1.0
            j0 = core * JPC + t * P
            pk_m[np.arange(P), t * N + t * P + np.arange(P)] = 1.0
            sel = (near_j >= j0) & (near_j < j0 + P)
            if sel.any():
                pk_m[near_j[sel] - j0,
                     t * N + (near_i[sel] - core * JPC) % N] = 1.0
            scal_m[:, 8 * t + 0] = r2[jj]
            scal_m[:, 8 * t + 1] = ar[jj]
            scal_m[:, 8 * t + 2] = ad[jj]
            scal_m[:, 8 * t + 3] = 1.6 * s0[jj]
            scal_m[:, 8 * t + 4] = 0.8 * s0[jj]
            scal_m[:, 8 * t + 5] = w0j[jj] / 3.0
            scal_m[:, 8 * t + 6] = w0j[jj] / 6.0
            scal_m[:, 8 * t + 7] = LN5
            u3 = qs[jj] * c[jj]
            u4 = qs[jj] * one_m2c[jj]
            wtsb_m[:, 8 * t + 0] = u3 * sfa[jj]
            wtsb_m[:, 8 * t + 1] = u4 * sfa[jj]
            wtsb_m[:, 8 * t + 2] = u3
            wtsb_m[:, 8 * t + 3] = u4
            wtsb_m[:, 8 * t + 4] = f16 * u3
            wtsb_m[:, 8 * t + 5] = f16 * u4
            wtsb_m[:, 8 * t + 6] = c[jj]
            wtsb_m[:, 8 * t + 7] = one_m2c[jj]

        in_maps.append(dict(
            geo=geo,
            brrow=rot(br).astype(f32)[None, :],
            bdrow=rot(bd).astype(f32)[None, :],
            scal=scal_m,
            wtsb=wtsb_m.astype(ml_dtypes.bfloat16),
            pkid=pkid_m,
            pk=pk_m.astype(ml_dtypes.bfloat16)))

    # exact (fp64) contributions of the poked near pairs
    e_elec_corr = 0.0
    e_vdw_corr = 0.0
    if len(near_i):
        X64f = np.asarray(inputs["X"], np.float32).astype(np.float64)
        m = c[near_i] != c[near_j]
        ia, ja = near_i[m], near_j[m]
        if len(ia):
            V = X64f[ja] - X64f[ia]
            D = np.sqrt((V * V).sum(1) + 3e-6)
            invD = 1.0 / (D + 1e-6)
            sfa64 = sfa.astype(np.float64); sfb64 = sfb.astype(np.float64)
            qs64 = qs.astype(np.float64)
            e_elec_corr = 0.5 * CONV * np.sum(
                qs64[ia] * qs64[ja] * invD
                * (sfa64[ja] + sfb64[ia] + float(f16) * invD))
            sig_r = 1.0 / (1.0 + np.exp(-(ar.astype(np.float64)[ja]
                                          + br.astype(np.float64)[ia])))
            s = 2.0 * s0.astype(np.float64)[ja] * (0.8 * sig_r + 0.4)
            repl = 5.0 * np.exp(-0.3 * D ** 3)
            Dm = D - s
            attr = (np.exp(-(Dm - 0.3) ** 2) + np.exp(-3.0 * Dm * Dm)
                    + np.exp(-10.0 * Dm * Dm)) / 3.0
            sig_d = 1.0 / (1.0 + np.exp(-(ad.astype(np.float64)[ja]
                                          + bd.astype(np.float64)[ia])))
            w = w0j.astype(np.float64)[ja] * (sig_d + 0.5)
            e_vdw_corr = np.sum(-w * attr + repl)
    aux = dict(qs=qs, c=c, sfb=sfb, inputs=inputs,
               e_elec_corr=e_elec_corr, e_vdw_corr=e_vdw_corr)
    return in_maps, aux


def _host_post(core_rows, aux):
    f64 = np.float64
    rows = np.zeros((8, N), f64)
    for core, r in enumerate(core_rows):
        r8 = np.concatenate([r[0:4], r[32:34], r[64:66]], axis=0)
        rows += np.roll(r8.astype(f64), core * JPC, axis=-1)
    qs = aux["qs"].astype(f64)
    c = aux["c"].astype(f64)
    sfb = aux["sfb"].astype(f64)
    R1, R2, R3, R4, R5, R6, V1, V2 = rows

    E_elec = 0.5 * CONV * np.sum(
        qs * (R1 + c * R2 + sfb * (R3 + c * R4) + R5 + c * R6))
    E_elec += aux["e_elec_corr"]
    E_vdw = np.sum(V1 + c * V2) + aux["e_vdw_corr"]

    inputs = aux["inputs"]
    embs = np.asarray(inputs["embs"], np.float32)
    die = np.asarray(inputs["die_factor"], np.float32)
    born = np.asarray(inputs["born_factor"], np.float32)
    qsf = np.asarray(inputs["qs"], np.float32).astype(f64)
    atomic_die = (embs @ die + 1e-6).astype(f64)
    R = (embs @ born + 1.0).astype(f64)
    E_self = -(1.0 - 1.0 / atomic_die) * qsf / (R + 1e-6)
    E_solv = CONV * np.sum(E_self) * 0.01

    def guard(e):
        return np.float32(1e-6) if np.isnan(e) else np.float32(e)

    return np.asarray([guard(E_vdw), guard(E_elec), guard(E_solv)],
                      dtype=np.float32)


def kernel(**inputs):
    nc = _build()
    in_maps, aux = _host_pre(inputs)
    res = run_bass_kernel_spmd(nc, in_maps, list(range(NCORES)))
    core_rows = [res.results[cid]["rows"] for cid in range(NCORES)]
    return _host_post(core_rows, aux)



if __name__ == "__main__":
    pass



# revision 13
# speedup vs baseline: 2.8025x; 2.8025x over previous
"""EnergyNet Trainium2 kernel v3 (SPMD over 8 NeuronCores).

Layout: partitions = j (each core owns 256 j's as 2 tiles of 128), free
dim = i. Each tile gets its OWN permutation of the i axis: columns
sorted by min-distance to the tile's 128 atoms (atoms are k-d ordered so
a 128-block's neighborhood is compact), so the 512-column prefix holds
every pair within the vdW cutoff. Electrostatics run full width; the
vdW chain (sigmoids + 3 Gaussians via Derivative_Erf + repulsion exp)
runs on shrinking prefixes (448/384/320/256).

D^2 comes from one k=13 fp16 hi/lo-split Gram matmul (PE multiplies
fp16 exactly, PSUM accumulates fp32; |err| ~ 5e-4). Near pairs
(D^2 < 0.25) and the diagonal get +1e6 pokes so their device
contribution is ~0 (elec) / exactly 0 (vdW); the host adds their exact
fp64 contributions.

Both reduction stages run on the PE: stage 1 uses the maps (invD,
invD2, vdw) as stationary lhsT against per-j weight columns, giving
per-i partials; stage 2 contracts those over i with per-atom weight
columns (qs, qs*c, qs*sfb, qs*sfb*c, 1, c), accumulating 6x6 / 2x6
energy cells in PSUM. The host combines 2*16*8 fp32 cells in fp64.
"""
import numpy as np
import ml_dtypes

import concourse.bass as bass
import concourse.mybir as mybir
import bass_rust as _bass_rust
from concourse.bass_utils import run_bass_kernel_spmd
from concourse.tile import TileContext

N = 2048
C = 8
CONV = 332.07156
NCORES = 8
P = 128
JT = 2          # j-tiles per core
WC = 512        # compact prefix (pokes + vdW support)
WEA = 448       # width for exp(-(Dm-0.3)^2) and the vdw map
WE3 = 384       # width for exp(-3 Dm^2)
WE10 = 384      # width for exp(-10 Dm^2); 3 full 128-blocks
WRP = 256       # width for repulsion 5 exp(-0.3 D^3)
CUT = 9.0       # neighbor cutoff (A) for the compact prefix
NEAR_TH2 = 0.25
POKE = 1.0e6
NB = 16         # stage-1 i-blocks of 128
LN5 = float(np.log(5.0))
SQ3 = float(np.sqrt(3.0))
SQ10 = float(np.sqrt(10.0))
SQPI = float(np.sqrt(np.pi))

AF = mybir.ActivationFunctionType
ALU = mybir.AluOpType
F32 = mybir.dt.float32
BF16 = mybir.dt.bfloat16
FP16 = mybir.dt.float16


# --------------------------------------------------------------- patches
def _patched_drain_and_barrier(self, tick_clock, wait_clock):
    gc = tick_clock.global_clock
    try:
        n_procs = len(gc)
    except TypeError:
        n_procs = 27
    ticks = [gc[p] for p in range(n_procs)]
    for p in [p for p in range(n_procs) if ticks[p] > 0] or [0]:
        d = self.nc.sync.drain()
        sub = [ticks[q] if q == p else 0 for q in range(n_procs)]
        wait_clock.add_sem_waits(
            d.ins, _bass_rust.ScopedClock({None: _bass_rust.VectorClock(sub)})
        )
    self.nc.all_engine_barrier()
    assert self.sems is not None
    popped = self.nc._tile_sem_poison_stack.pop()
    assert popped is self._sem_poison
    self.nc.clear_and_free_semaphores(list(self.sems.allocated().values()))
    self.nc.all_engine_barrier()


TileContext._drain_and_barrier = _patched_drain_and_barrier

_NOPC = [0]


def _split_excess_waits(nc):
    """This walrus build rejects instructions carrying more than one sem
    wait. Hoist excess waits onto same-engine NoOps inserted just before
    the offending instruction (the engine sequencer executes them in
    order, so the waits still gate it)."""
    for blk in nc.m.functions[0].blocks:
        insts = blk.instructions
        out = []
        changed = False
        for inst in insts:
            si = inst.sync_info
            waits = list(si.on_wait) if si is not None else []
            if len(waits) > 1:
                keep_idx = len(waits) - 1
                if type(inst).__name__ == "InstDMACopy":
                    for k, w in enumerate(waits):
                        if str(getattr(w, "ant_name", "")).startswith(
                                ("DMAHW", "DMASW")):
                            keep_idx = k
                            break
                rest = [w for k, w in enumerate(waits) if k != keep_idx]
                for w in rest:
                    _NOPC[0] += 1
                    nop = mybir.InstNoOp(name=f"WH-{_NOPC[0]}", ins=[], outs=[])
                    nop.engine = inst.engine
                    nop.sync_info = mybir.SyncInfo(on_wait=[w], on_update=[])
                    out.append(nop)
                inst.sync_info = mybir.SyncInfo(on_wait=[waits[keep_idx]],
                                                on_update=list(si.on_update))
                changed = True
            out.append(inst)
        if changed:
            blk.instructions = out


def _bcast_src(dram_ap, n_free):
    """Stride-0 partition AP: read one DRAM row into all 128 partitions."""
    return bass.AP(tensor=dram_ap.tensor, offset=0,
                   ap=_bass_rust.VecI64Pair([[0, P], [1, n_free]]))


_CACHE = {}


def _build():
    if "nc" in _CACHE:
        return _CACHE["nc"]
    nc = bass.Bass()
    geo = [nc.declare_dram_parameter(f"geo{t}", [13, N + P], FP16,
                                     isOutput=False) for t in range(JT)]
    pks = nc.declare_dram_parameter("pks", [P, P + 2 * WC], BF16,
                                    isOutput=False)
    bc = nc.declare_dram_parameter("bc", [1, 4 * WC], FP16, isOutput=False)
    scw = nc.declare_dram_parameter("scw", [P, 8 * JT + 6 * NB * JT], F32,
                                    isOutput=False)
    wts = nc.declare_dram_parameter("wts", [P, 8 * JT], FP16, isOutput=False)
    cells_out = nc.declare_dram_parameter("cells", [66, 12], F32,
                                          isOutput=True)

    with TileContext(nc) as tc:
        with tc.tile_pool(name="const", bufs=1) as cpool, \
             tc.tile_pool(name="work", bufs=1) as wpool, \
             tc.tile_pool(name="gpin", bufs=2, space="PSUM") as gpin, \
             tc.tile_pool(name="gring", bufs=2, space="PSUM") as gring, \
             tc.tile_pool(name="pvals", bufs=1, space="PSUM") as pvals, \
             tc.tile_pool(name="pout", bufs=1, space="PSUM") as poutp:

            t_geofull = [cpool.tile([13, N + P], FP16, name=f"t_geo{t}")
                         for t in range(JT)]
            t_geo = [g[:, 0:N] for g in t_geofull]
            t_geoT = [g[:, N:N + P] for g in t_geofull]
            t_pks = cpool.tile([P, P + 2 * WC], BF16, name="t_pks")
            t_pkid = t_pks[:, 0:P]
            t_pk = [t_pks[:, P + t * WC:P + (t + 1) * WC] for t in range(JT)]
            t_scw = cpool.tile([P, 8 * JT + 6 * NB * JT], F32, name="t_scw")
            t_scal = t_scw[:, 0:8 * JT]
            t_wq = t_scw[:, 8 * JT:]
            t_wts = cpool.tile([P, 8 * JT], FP16, name="t_wts")
            t_bc = cpool.tile([P, 4 * WC], FP16, name="t_bc")
            t_brc = [t_bc[:, 2 * WC * t:2 * WC * t + WC] for t in range(JT)]
            t_bdc = [t_bc[:, 2 * WC * t + WC:2 * WC * (t + 1)]
                     for t in range(JT)]
            # DMA order = need order
            nc.sync.dma_start(t_scw[:], scw[:])
            nc.sync.dma_start(t_geofull[0][:], geo[0][:])
            nc.sync.dma_start(t_pks[:], pks[:])
            nc.sync.dma_start(t_bc[:], _bcast_src(bc[:], 4 * WC))
            nc.sync.dma_start(t_geofull[1][:], geo[1][:])
            nc.sync.dma_start(t_wts[:], wts[:])

            # out cells (matmul col base must be 0/32/64)
            t_out_e = poutp.tile([38, 6], F32, name="t_out_e")
            t_out_v = poutp.tile([66, 12], F32, name="t_out_v")

            def sc(t, k):
                return t_scal[:, 8 * t + k:8 * t + k + 1]

            # ---- sigmoids first (independent of the Gram chain)
            sigr, sigd = [], []
            for t in range(JT):
                sr = wpool.tile([P, WEA], FP16, name=f"sigr_{t}")
                nc.scalar.activation(sr[:], t_brc[t][:, 0:WEA], AF.Sigmoid,
                                     bias=sc(t, 0))
                sd = wpool.tile([P, WEA], FP16, name=f"sigd_{t}")
                nc.scalar.activation(sd[:], t_bdc[t][:, 0:WEA], AF.Sigmoid,
                                     bias=sc(t, 1))
                sigr.append(sr)
                sigd.append(sd)

            # ---- per tile: Gram -> invD2 -> invD -> D_c -> Dm
            invD, invD2, D_c, Dm = [], [], [], []
            for t in range(JT):
                Gb = []
                for cidx in range(4):
                    if cidx == 0:
                        g = gpin.tile([P, 512], F32, name=f"G_{t}_0",
                                      tag="Gpin")
                    else:
                        g = gring.tile([P, 512], F32, name=f"G_{t}_{cidx}",
                                       tag="G")
                    cs = 512 * cidx
                    nc.tensor.matmul(g[:], t_geoT[t][:],
                                     t_geo[t][0:13, cs:cs + 512],
                                     start=True, stop=(cidx != 0))
                    if cidx == 0:
                        nc.tensor.matmul(g[:], t_pkid[:], t_pk[t][:],
                                         start=False, stop=True)
                    Gb.append(g)

                iD2 = wpool.tile([P, N], FP16, name=f"invD2_{t}")
                with nc.allow_low_precision(reason="fp16 maps; reductions "
                                            "accumulate fp32 in PSUM"):
                    for cidx in range(4):
                        sl = slice(cidx * 512, (cidx + 1) * 512)
                        nc.vector.reciprocal(iD2[:, sl], Gb[cidx][:])
                iD = wpool.tile([P, N], FP16, name=f"invD_{t}")
                for h in range(2):
                    sl = slice(h * 1024, (h + 1) * 1024)
                    nc.scalar.activation(iD[:, sl], iD2[:, sl], AF.Sqrt)
                invD2.append(iD2)
                invD.append(iD)

                # D_c needs only invD[:, 0:WEA] (first sqrt half) + G0
                dc = wpool.tile([P, WEA], FP16, name=f"Dc_{t}")
                nc.vector.tensor_tensor(dc[:], Gb[0][:, 0:WEA],
                                        iD[:, 0:WEA], ALU.mult)
                s_m = wpool.tile([P, WEA], FP16, name=f"s_{t}")
                nc.vector.tensor_scalar(s_m[:], sigr[t][:], sc(t, 2),
                                        sc(t, 3), ALU.mult, ALU.add)
                dm = wpool.tile([P, WEA], FP16, name=f"Dm_{t}")
                nc.vector.tensor_tensor(dm[:], dc[:], s_m[:], ALU.subtract)
                D_c.append(dc)
                Dm.append(dm)

            # ---- per tile: Gaussians + repulsion + vdw map + reductions
            for t in range(JT):
                ea = wpool.tile([P, WEA], BF16, name=f"ea_{t}")
                nc.scalar.activation(ea[:], Dm[t][:], AF.Derivative_Erf,
                                     bias=sc(t, 6))
                eb = wpool.tile([P, WE3], BF16, name=f"eb_{t}")
                nc.scalar.activation(eb[:], Dm[t][:, 0:WE3],
                                     AF.Derivative_Erf, scale=SQ3)
                ec = wpool.tile([P, WE10], BF16, name=f"ec_{t}")
                nc.scalar.activation(ec[:], Dm[t][:, 0:WE10],
                                     AF.Derivative_Erf, scale=SQ10)
                nc.gpsimd.tensor_tensor(ea[:, 0:WE3], ea[:, 0:WE3], eb[:],
                                        ALU.add)
                w3 = wpool.tile([P, WEA], BF16, name=f"w3_{t}")
                nc.vector.tensor_scalar(w3[:], sigd[t][:], sc(t, 4), sc(t, 5),
                                        ALU.mult, ALU.add)
                WS = wpool.tile([P, WC], BF16, name=f"WS_{t}")
                nc.gpsimd.memset(WS[:, WEA:WC], 0.0)
                nc.vector.tensor_tensor(WS[:, 0:WEA], w3[:], ea[:], ALU.mult)
                WE = wpool.tile([P, WE10], BF16, name=f"WE_{t}")
                nc.vector.tensor_tensor(WE[:], w3[:, 0:WE10], ec[:], ALU.mult)

                D2c = wpool.tile([P, WRP], BF16, name=f"D2c_{t}")
                nc.gpsimd.tensor_tensor(D2c[:], D_c[t][:, 0:WRP],
                                        D_c[t][:, 0:WRP], ALU.mult)
                D3 = wpool.tile([P, WRP], BF16, name=f"D3_{t}")
                nc.gpsimd.tensor_tensor(D3[:], D2c[:], D_c[t][:, 0:WRP],
                                        ALU.mult)
                repl = wpool.tile([P, WRP], BF16, name=f"repl_{t}")
                nc.scalar.activation(repl[:], D3[:], AF.Exp, scale=-0.3,
                                     bias=sc(t, 7))

                # ---- stage 1: per-i partials (maps as stationary lhsT)
                # vals: elec 6/block (0:96), vdw 2/block: WS 4 blocks
                # (96:104), WE 3 blocks (104:110), repl 2 blocks (110:114)
                vals = pvals.tile([P, 114], F32, name=f"vals_{t}",
                                  tag="vals")
                for b in range(NB):
                    bl = slice(b * P, (b + 1) * P)
                    nc.tensor.matmul(vals[:, 6 * b:6 * b + 4],
                                     invD[t][:, bl], t_wts[:, 8 * t:8 * t + 4],
                                     start=True, stop=True)
                    nc.tensor.matmul(vals[:, 6 * b + 4:6 * b + 6],
                                     invD2[t][:, bl],
                                     t_wts[:, 8 * t + 2:8 * t + 4],
                                     start=True, stop=True)
                for b in range(4):
                    nc.tensor.matmul(vals[:, 96 + 2 * b:96 + 2 * b + 2],
                                     WS[:, b * P:(b + 1) * P],
                                     t_wts[:, 8 * t + 4:8 * t + 6],
                                     start=True, stop=True)
                for b in range(3):
                    nc.tensor.matmul(vals[:, 104 + 2 * b:104 + 2 * b + 2],
                                     WE[:, b * P:(b + 1) * P],
                                     t_wts[:, 8 * t + 4:8 * t + 6],
                                     start=True, stop=True)
                for b in range(2):
                    nc.tensor.matmul(vals[:, 110 + 2 * b:110 + 2 * b + 2],
                                     repl[:, b * P:(b + 1) * P],
                                     t_wts[:, 8 * t + 6:8 * t + 8],
                                     start=True, stop=True)

                svals = wpool.tile([P, 114], F32, name=f"sv_{t}")
                nc.vector.tensor_scalar(svals[:], vals[:], 1.0, None,
                                        ALU.mult)

                # ---- stage 2: contract over i with per-atom weights
                for b in range(NB):
                    nc.tensor.matmul(t_out_e[32 * t:32 * t + 6, :],
                                     svals[:, 6 * b:6 * b + 6],
                                     t_wq[:, 6 * NB * t + 6 * b:
                                          6 * NB * t + 6 * b + 6],
                                     start=(b == 0), stop=(b == NB - 1))
                v2 = ([(96 + 2 * b, b) for b in range(4)]
                      + [(104 + 2 * b, b) for b in range(3)]
                      + [(110 + 2 * b, b) for b in range(2)])
                for k, (col, b) in enumerate(v2):
                    nc.tensor.matmul(t_out_v[64:66, 6 * t:6 * t + 6],
                                     svals[:, col:col + 2],
                                     t_wq[:, 6 * NB * t + 6 * b:
                                          6 * NB * t + 6 * b + 6],
                                     start=(k == 0), stop=(k == len(v2) - 1))

            # ---- evacuate the cell groups (partition-aligned copies into
            # one zeroed SBUF tile, then a single DMA)
            sb_out = wpool.tile([66, 12], F32, name="sb_out")
            nc.gpsimd.memset(sb_out[:], 0.0)
            for t in range(JT):
                nc.vector.tensor_scalar(sb_out[32 * t:32 * t + 6, 0:6],
                                        t_out_e[32 * t:32 * t + 6, :],
                                        1.0, None, ALU.mult)
            nc.vector.tensor_scalar(sb_out[64:66, :],
                                    t_out_v[64:66, :],
                                    1.0, None, ALU.mult)
            nc.sync.dma_start(cells_out[:], sb_out[:])

    _split_excess_waits(nc)
    _CACHE["nc"] = nc
    return nc


# --------------------------------------------------------------- host side
def _kd_order(X):
    out = []

    def rec(ids):
        if len(ids) <= P:
            out.append(ids)
            return
        spans = X[ids].max(0) - X[ids].min(0)
        ax = int(np.argmax(spans))
        order = ids[np.argsort(X[ids, ax], kind="stable")]
        half = (len(ids) // 2 // P) * P
        rec(order[:half])
        rec(order[half:])

    rec(np.arange(len(X)))
    return np.concatenate(out)


def _f16_split(x):
    h = x.astype(np.float16)
    l = (x - h.astype(np.float64)).astype(np.float16)
    return h, l


def _host_pre(inputs):
    f32, f64 = np.float32, np.float64
    X = np.asarray(inputs["X"], f32)
    embs = np.asarray(inputs["embs"], f32)
    qs = np.asarray(inputs["qs"], f32)
    w0 = np.asarray(inputs["w0"], f32)
    s0 = np.asarray(inputs["s0"], f32)
    c = np.asarray(inputs["chainidx"]).astype(f32)
    f = np.asarray(inputs["sf_elec"], f32)[:, 0]
    rf = np.asarray(inputs["radius_factor"], f32)[:, 0]
    df = np.asarray(inputs["depth_factor"], f32)[:, 0]

    X64 = X.astype(f64)
    Xc64 = X64 - X64.mean(0)
    r2_64 = (Xc64 ** 2).sum(1)
    D2x = r2_64[:, None] + r2_64[None, :] - 2.0 * (Xc64 @ Xc64.T)
    np.fill_diagonal(D2x, 0.0)
    D2x = np.maximum(D2x, 0.0)

    perm = _kd_order(Xc64)

    # sorted-frame quantities
    r2s = r2_64[perm]
    D2s = D2x[perm][:, perm]
    Xs = Xc64[perm]
    sfa = (embs @ f[:C]).astype(f64)[perm]
    sfb = (embs @ f[C:2 * C]).astype(f64)[perm]
    f16 = float(f[2 * C])
    ar = (embs @ rf[:C]).astype(f64)[perm]
    br = (embs @ rf[C:]).astype(f64)[perm]
    ad = (embs @ df[:C]).astype(f64)[perm]
    bd = (embs @ df[C:]).astype(f64)[perm]
    w0j = np.sqrt(w0.astype(f64) ** 2 + 1e-6)[perm]
    qs_s = qs.astype(f64)[perm]
    c_s = c.astype(f64)[perm]
    s0_s = s0.astype(f64)[perm]

    hx, lx = _f16_split(Xs)
    hr2j, lr2j = _f16_split(r2s)
    hr2i, lr2i = _f16_split(r2s + 3e-6)

    pkid_m = (np.eye(P, dtype=f32) * POKE).astype(ml_dtypes.bfloat16)
    u3 = qs_s * c_s
    u4 = qs_s * (1.0 - 2.0 * c_s)

    in_maps = []
    for core in range(NCORES):
        m = {}
        pks_m = np.zeros((P, P + 2 * WC), f32)
        pks_m[:, 0:P] = np.eye(P, dtype=f32) * POKE
        bc_m = np.zeros((1, 4 * WC), np.float16)
        scal_m = np.zeros((P, 8 * JT), f32)
        wts_m = np.zeros((P, 8 * JT), f32)
        wq_m = np.zeros((P, 6 * NB * JT), f32)
        for t in range(JT):
            g0 = core * (P * JT) + t * P
            jj = slice(g0, g0 + P)
            minD2 = D2s[jj].min(0)
            pi = np.argsort(minD2, kind="stable")   # full 2048 permutation

            geo_m = np.zeros((13, N + P), np.float16)
            geo_r = geo_m[:, 0:N]
            geoT_m = geo_m[:, N:N + P]
            for d in range(3):
                geo_r[3 * d + 0] = hx[pi, d]
                geo_r[3 * d + 1] = lx[pi, d]
                geo_r[3 * d + 2] = hx[pi, d]
                geoT_m[3 * d + 0] = -2.0 * hx[jj, d]
                geoT_m[3 * d + 1] = -2.0 * hx[jj, d]
                geoT_m[3 * d + 2] = -2.0 * lx[jj, d]
            geo_r[9] = 1.0
            geo_r[10] = 1.0
            geoT_m[9] = hr2j[jj]
            geoT_m[10] = lr2j[jj]
            geo_r[11] = hr2i[pi]
            geo_r[12] = lr2i[pi]
            geoT_m[11] = 1.0
            geoT_m[12] = 1.0

            pos = np.empty(N, np.int64)
            pos[pi] = np.arange(N)
            pk_m = np.zeros((P, WC), f32)
            pk_m[np.arange(P), pos[g0 + np.arange(P)]] = POKE
            nj, ni_ = np.where(D2s[jj] < NEAR_TH2)
            sel = ni_ != (g0 + nj)
            pk_m[nj[sel], pos[ni_[sel]]] = POKE

            m[f"geo{t}"] = geo_m
            pks_m[:, P + t * WC:P + (t + 1) * WC] = pk_m / POKE
            bc_m[0, 2 * WC * t:2 * WC * t + WC] = br[pi[:WC]].astype(
                np.float16)
            bc_m[0, 2 * WC * t + WC:2 * WC * (t + 1)] = bd[pi[:WC]].astype(
                np.float16)

            scal_m[:, 8 * t + 0] = ar[jj]
            scal_m[:, 8 * t + 1] = ad[jj]
            scal_m[:, 8 * t + 2] = 1.6 * s0_s[jj]
            scal_m[:, 8 * t + 3] = 0.8 * s0_s[jj]
            scal_m[:, 8 * t + 4] = w0j[jj] * (SQPI / 6.0)
            scal_m[:, 8 * t + 5] = w0j[jj] * (SQPI / 12.0)
            scal_m[:, 8 * t + 6] = -0.3
            scal_m[:, 8 * t + 7] = LN5
            wts_m[:, 8 * t + 0] = u3[jj] * sfa[jj]
            wts_m[:, 8 * t + 1] = u4[jj] * sfa[jj]
            wts_m[:, 8 * t + 2] = u3[jj]
            wts_m[:, 8 * t + 3] = u4[jj]
            wts_m[:, 8 * t + 4] = c_s[jj]
            wts_m[:, 8 * t + 5] = 1.0 - 2.0 * c_s[jj]
            wts_m[:, 8 * t + 6] = -c_s[jj]
            wts_m[:, 8 * t + 7] = -(1.0 - 2.0 * c_s[jj])
            for b in range(NB):
                ib = pi[b * P:(b + 1) * P]
                base = 6 * NB * t + 6 * b
                wq_m[:, base + 0] = qs_s[ib]
                wq_m[:, base + 1] = qs_s[ib] * c_s[ib]
                wq_m[:, base + 2] = qs_s[ib] * sfb[ib]
                wq_m[:, base + 3] = qs_s[ib] * sfb[ib] * c_s[ib]
                wq_m[:, base + 4] = 1.0
                wq_m[:, base + 5] = c_s[ib]
        m["pks"] = pks_m.astype(ml_dtypes.bfloat16)
        m["bc"] = bc_m
        scw_m = np.concatenate([scal_m, wq_m], axis=1)
        m["scw"] = scw_m
        m["wts"] = wts_m.astype(np.float16)
        in_maps.append(m)

    # ---- exact fp64 contributions of the poked near pairs (device ~0)
    ni_a, nj_a = np.where((D2s < NEAR_TH2) & (D2s > 0))
    e_elec_corr = 0.0
    e_vdw_corr = 0.0
    if len(ni_a):
        msk = c_s[ni_a] != c_s[nj_a]
        ia, ja = ni_a[msk], nj_a[msk]       # ordered pairs, both directions
        Dn = np.sqrt(D2s[ia, ja] + 3e-6)
        invDn = 1.0 / (Dn + 1e-6)
        e_elec_corr = 0.5 * CONV * np.sum(
            qs_s[ia] * qs_s[ja] * invDn
            * (sfa[ja] + sfb[ia] + f16 * invDn))
        sig_r = 1.0 / (1.0 + np.exp(-(ar[ja] + br[ia])))
        s = 2.0 * s0_s[ja] * (0.8 * sig_r + 0.4)
        repl = 5.0 * np.exp(-0.3 * Dn ** 3)
        Dmn = Dn - s
        attr = (np.exp(-(Dmn - 0.3) ** 2) + np.exp(-3.0 * Dmn * Dmn)
                + np.exp(-10.0 * Dmn * Dmn)) / 3.0
        sig_d = 1.0 / (1.0 + np.exp(-(ad[ja] + bd[ia])))
        w = w0j[ja] * (sig_d + 0.5)
        e_vdw_corr = np.sum(-w * attr + repl)

    aux = dict(inputs=inputs, f16=f16,
               e_elec_corr=e_elec_corr, e_vdw_corr=e_vdw_corr)
    return in_maps, aux


def _host_post(core_cells, aux):
    f64 = np.float64
    f16 = aux["f16"]
    E_elec = 0.0
    E_vdw = 0.0
    for cells in core_cells:
        cc = cells.astype(f64)
        for t in range(JT):
            e = cc[32 * t:32 * t + 6, 0:6]
            v = cc[64:66, 6 * t:6 * t + 6]
            E_elec += (e[0, 0] + e[1, 1] + e[2, 2] + e[3, 3]
                       + f16 * (e[4, 0] + e[5, 1]))
            E_vdw += -(v[0, 4] + v[1, 5])
    E_elec = 0.5 * CONV * E_elec + aux["e_elec_corr"]
    E_vdw = E_vdw + aux["e_vdw_corr"]

    inputs = aux["inputs"]
    embs = np.asarray(inputs["embs"], np.float32)
    die = np.asarray(inputs["die_factor"], np.float32)
    born = np.asarray(inputs["born_factor"], np.float32)
    qsf = np.asarray(inputs["qs"], np.float32).astype(f64)
    atomic_die = (embs @ die + 1e-6).astype(f64)
    R = (embs @ born + 1.0).astype(f64)
    E_self = -(1.0 - 1.0 / atomic_die) * qsf / (R + 1e-6)
    E_solv = CONV * np.sum(E_self) * 0.01

    def guard(e):
        return np.float32(1e-6) if np.isnan(e) else np.float32(e)

    return np.asarray([guard(E_vdw), guard(E_elec), guard(E_solv)],
                      dtype=np.float32)


def kernel(**inputs):
    nc = _build()
    in_maps, aux = _host_pre(inputs)
    res = run_bass_kernel_spmd(nc, in_maps, list(range(NCORES)))
    core_cells = [res.results[cid]["cells"] for cid in range(NCORES)]
    return _host_post(core_cells, aux)


if __name__ == "__main__":
    pass


# revision 21
# speedup vs baseline: 2.9542x; 1.0541x over previous
"""EnergyNet Trainium2 kernel v3 (SPMD over 8 NeuronCores).

Layout: partitions = j (each core owns 256 j's as 2 tiles of 128), free
dim = i. Each tile gets its OWN permutation of the i axis: columns
sorted by min-distance to the tile's 128 atoms (atoms are k-d ordered so
a 128-block's neighborhood is compact), so the 512-column prefix holds
every pair within the vdW cutoff. Electrostatics run full width; the
vdW chain (sigmoids + 3 Gaussians via Derivative_Erf + repulsion exp)
runs on shrinking prefixes (448/384/320/256).

D^2 comes from one k=13 fp16 hi/lo-split Gram matmul (PE multiplies
fp16 exactly, PSUM accumulates fp32; |err| ~ 5e-4). Near pairs
(D^2 < 0.25) and the diagonal get +1e6 pokes so their device
contribution is ~0 (elec) / exactly 0 (vdW); the host adds their exact
fp64 contributions.

Both reduction stages run on the PE: stage 1 uses the maps (invD,
invD2, vdw) as stationary lhsT against per-j weight columns, giving
per-i partials; stage 2 contracts those over i with per-atom weight
columns (qs, qs*c, qs*sfb, qs*sfb*c, 1, c), accumulating 6x6 / 2x6
energy cells in PSUM. The host combines 2*16*8 fp32 cells in fp64.
"""
import numpy as np
import ml_dtypes

import concourse.bass as bass
import concourse.mybir as mybir
import bass_rust as _bass_rust
from concourse.bass_utils import run_bass_kernel_spmd
from concourse.tile import TileContext

N = 2048
C = 8
CONV = 332.07156
NCORES = 8
P = 128
JT = 2          # j-tiles per core
WC = 512        # compact prefix (pokes + vdW support)
WEA = 448       # width for exp(-(Dm-0.3)^2) and the vdw map
WE3 = 384       # width for exp(-3 Dm^2)
WE10 = 384      # width for exp(-10 Dm^2); 3 full 128-blocks
WRP = 256       # width for repulsion 5 exp(-0.3 D^3)
CUT = 9.0       # neighbor cutoff (A) for the compact prefix
NEAR_TH2 = 0.25
POKE = 1.0e6
NB = 16         # stage-1 i-blocks of 128
LN5 = float(np.log(5.0))
SQ3 = float(np.sqrt(3.0))
SQ10 = float(np.sqrt(10.0))
SQPI = float(np.sqrt(np.pi))

AF = mybir.ActivationFunctionType
ALU = mybir.AluOpType
F32 = mybir.dt.float32
BF16 = mybir.dt.bfloat16
FP16 = mybir.dt.float16


# --------------------------------------------------------------- patches
def _patched_drain_and_barrier(self, tick_clock, wait_clock):
    gc = tick_clock.global_clock
    try:
        n_procs = len(gc)
    except TypeError:
        n_procs = 27
    ticks = [gc[p] for p in range(n_procs)]
    for p in [p for p in range(n_procs) if ticks[p] > 0] or [0]:
        d = self.nc.sync.drain()
        sub = [ticks[q] if q == p else 0 for q in range(n_procs)]
        wait_clock.add_sem_waits(
            d.ins, _bass_rust.ScopedClock({None: _bass_rust.VectorClock(sub)})
        )
    self.nc.all_engine_barrier()
    assert self.sems is not None
    popped = self.nc._tile_sem_poison_stack.pop()
    assert popped is self._sem_poison
    self.nc.clear_and_free_semaphores(list(self.sems.allocated().values()))


TileContext._drain_and_barrier = _patched_drain_and_barrier

_NOPC = [0]


def _split_excess_waits(nc):
    """This walrus build rejects instructions carrying more than one sem
    wait. Hoist excess waits onto same-engine NoOps inserted just before
    the offending instruction (the engine sequencer executes them in
    order, so the waits still gate it)."""
    for blk in nc.m.functions[0].blocks:
        insts = blk.instructions
        out = []
        changed = False
        for inst in insts:
            si = inst.sync_info
            waits = list(si.on_wait) if si is not None else []
            if len(waits) > 1:
                keep_idx = len(waits) - 1
                if type(inst).__name__ == "InstDMACopy":
                    for k, w in enumerate(waits):
                        if str(getattr(w, "ant_name", "")).startswith(
                                ("DMAHW", "DMASW")):
                            keep_idx = k
                            break
                rest = [w for k, w in enumerate(waits) if k != keep_idx]
                for w in rest:
                    _NOPC[0] += 1
                    nop = mybir.InstNoOp(name=f"WH-{_NOPC[0]}", ins=[], outs=[])
                    nop.engine = inst.engine
                    nop.sync_info = mybir.SyncInfo(on_wait=[w], on_update=[])
                    out.append(nop)
                inst.sync_info = mybir.SyncInfo(on_wait=[waits[keep_idx]],
                                                on_update=list(si.on_update))
                changed = True
            out.append(inst)
        if changed:
            blk.instructions = out


def _bcast_src(dram_ap, n_free):
    """Stride-0 partition AP: read one DRAM row into all 128 partitions."""
    return bass.AP(tensor=dram_ap.tensor, offset=0,
                   ap=_bass_rust.VecI64Pair([[0, P], [1, n_free]]))


_CACHE = {}


def _build():
    if "nc" in _CACHE:
        return _CACHE["nc"]
    nc = bass.Bass()
    # geo cols: [0:N rhs | N:N+P lhsT | +P one-hot lhsT | +2*WEA br/bd packs]
    GEOW = N + P + P + 2 * WEA
    geo = [nc.declare_dram_parameter(f"geo{t}", [13, GEOW], FP16,
                                     isOutput=False) for t in range(JT)]
    pks = nc.declare_dram_parameter("pks", [P, P + 2 * WC], BF16,
                                    isOutput=False)
    scal_d = nc.declare_dram_parameter("scal", [P, 8 * JT], F32,
                                       isOutput=False)
    scw = nc.declare_dram_parameter("scw", [P, 6 * NB * JT], F32,
                                    isOutput=False)
    wts = nc.declare_dram_parameter("wts", [P, 8 * JT], FP16, isOutput=False)
    cells_out = nc.declare_dram_parameter("cells", [66, 18], F32,
                                          isOutput=True)

    with TileContext(nc) as tc:
        with tc.tile_pool(name="const", bufs=1) as cpool, \
             tc.tile_pool(name="work", bufs=1) as wpool, \
             tc.tile_pool(name="gpin", bufs=2, space="PSUM") as gpin, \
             tc.tile_pool(name="gring", bufs=2, space="PSUM") as gring, \
             tc.tile_pool(name="pvals", bufs=1, space="PSUM") as pvals, \
             tc.tile_pool(name="pbc", bufs=1, space="PSUM") as pbc, \
             tc.tile_pool(name="pout", bufs=1, space="PSUM") as poutp:

            t_geofull = [cpool.tile([13, N + P + P + 2 * WEA], FP16,
                                    name=f"t_geo{t}") for t in range(JT)]
            t_geo = [g[:, 0:N] for g in t_geofull]
            t_geoT = [g[:, N:N + P] for g in t_geofull]
            t_bone = [g[:, N + P:N + 2 * P] for g in t_geofull]
            t_bpack = [g[:, N + 2 * P:] for g in t_geofull]
            t_pks = cpool.tile([P, P + 2 * WC], BF16, name="t_pks")
            t_pkid = t_pks[:, 0:P]
            t_pk = [t_pks[:, P + t * WC:P + (t + 1) * WC] for t in range(JT)]
            t_scal_t = cpool.tile([P, 8 * JT], F32, name="t_scal")
            t_scal = t_scal_t[:, :]
            t_scw = cpool.tile([P, 6 * NB * JT], F32, name="t_scw")
            t_wq = t_scw[:, :]
            t_wts = cpool.tile([P, 8 * JT], FP16, name="t_wts")
            # DMA order = need order
            nc.sync.dma_start(t_geofull[0][:], geo[0][:])
            nc.sync.dma_start(t_scal_t[:], scal_d[:])
            nc.sync.dma_start(t_pks[:], pks[:])
            nc.sync.dma_start(t_geofull[1][:], geo[1][:])
            nc.sync.dma_start(t_scw[:], scw[:])
            nc.sync.dma_start(t_wts[:], wts[:])

            # out cells (matmul col base must be 0/32/64):
            # elec t@[32t:32t+6, 0:6], vdw t@[64:66, 6+6t:12+6t]
            t_out = poutp.tile([66, 18], F32, name="t_out")
            t_out_e = t_out[:, 0:6]
            t_out_v = t_out[:, 0:18]

            def sc(t, k):
                return t_scal[:, 8 * t + k:8 * t + k + 1]

            # ---- sigmoid args via k=3 PE matmul: br_i/bd_i data row plus
            # ar_j/ad_j bias rows against segment indicators; one bias-free
            # sigmoid over both segments reads the PSUM directly
            sigr, sigd = [], []
            for t in range(JT):
                pb = pbc.tile([P, 2 * WEA], F32, name=f"bc_{t}", tag="bc")
                nc.tensor.matmul(pb[:, 0:WEA], t_bone[t][0:3, :],
                                 t_bpack[t][0:3, 0:WEA],
                                 start=True, stop=True)
                nc.tensor.matmul(pb[:, WEA:2 * WEA], t_bone[t][0:3, :],
                                 t_bpack[t][0:3, WEA:2 * WEA],
                                 start=True, stop=True)
                sg = wpool.tile([P, 2 * WEA], FP16, name=f"sigs_{t}")
                nc.scalar.activation(sg[:], pb[:], AF.Sigmoid)
                sigr.append(sg[:, 0:WEA])
                sigd.append(sg[:, WEA:2 * WEA])

            # ---- per tile: Gram -> invD2 -> invD -> D_c -> Dm
            invD, invD2, D_c, Dm = [], [], [], []
            for t in range(JT):
                Gb = []
                for cidx in range(4):
                    if cidx == 0:
                        g = gpin.tile([P, 512], F32, name=f"G_{t}_0",
                                      tag="Gpin")
                    else:
                        g = gring.tile([P, 512], F32, name=f"G_{t}_{cidx}",
                                       tag="G")
                    cs = 512 * cidx
                    nc.tensor.matmul(g[:], t_geoT[t][:],
                                     t_geo[t][0:13, cs:cs + 512],
                                     start=True, stop=(cidx != 0))
                    if cidx == 0:
                        nc.tensor.matmul(g[:], t_pkid[:], t_pk[t][:],
                                         start=False, stop=True)
                    Gb.append(g)

                iD2 = wpool.tile([P, N], FP16, name=f"invD2_{t}")
                with nc.allow_low_precision(reason="fp16 maps; reductions "
                                            "accumulate fp32 in PSUM"):
                    for cidx in range(4):
                        sl = slice(cidx * 512, (cidx + 1) * 512)
                        nc.vector.reciprocal(iD2[:, sl], Gb[cidx][:])
                iD = wpool.tile([P, N], FP16, name=f"invD_{t}")
                nc.scalar.activation(iD[:], iD2[:], AF.Sqrt)
                invD2.append(iD2)
                invD.append(iD)

                # D_c needs only invD[:, 0:WEA] (first sqrt half) + G0
                dc = wpool.tile([P, WEA], FP16, name=f"Dc_{t}")
                nc.vector.tensor_tensor(dc[:], Gb[0][:, 0:WEA],
                                        iD[:, 0:WEA], ALU.mult)
                s_m = wpool.tile([P, WEA], FP16, name=f"s_{t}")
                nc.vector.tensor_scalar(s_m[:], sigr[t][:], sc(t, 2),
                                        sc(t, 3), ALU.mult, ALU.add)
                dm = wpool.tile([P, WEA], FP16, name=f"Dm_{t}")
                nc.vector.tensor_tensor(dm[:], dc[:], s_m[:], ALU.subtract)
                D_c.append(dc)
                Dm.append(dm)

            # ---- per tile: Gaussians + repulsion + vdw map + reductions
            for t in range(JT):
                ea = wpool.tile([P, WEA], BF16, name=f"ea_{t}")
                nc.scalar.activation(ea[:], Dm[t][:], AF.Derivative_Erf,
                                     bias=sc(t, 6))
                eb = wpool.tile([P, WE3], BF16, name=f"eb_{t}")
                nc.scalar.activation(eb[:], Dm[t][:, 0:WE3],
                                     AF.Derivative_Erf, scale=SQ3)
                ec = wpool.tile([P, WE10], BF16, name=f"ec_{t}")
                nc.scalar.activation(ec[:], Dm[t][:, 0:WE10],
                                     AF.Derivative_Erf, scale=SQ10)
                w3 = wpool.tile([P, WEA], BF16, name=f"w3_{t}")
                nc.vector.tensor_scalar(w3[:], sigd[t][:], sc(t, 4), sc(t, 5),
                                        ALU.mult, ALU.add)
                WS = wpool.tile([P, WC], BF16, name=f"WS_{t}")
                nc.gpsimd.memset(WS[:, WEA:WC], 0.0)
                nc.vector.tensor_tensor(WS[:, 0:WEA], w3[:], ea[:], ALU.mult)
                WB = wpool.tile([P, WE3], BF16, name=f"WB_{t}")
                nc.vector.tensor_tensor(WB[:], w3[:, 0:WE3], eb[:], ALU.mult)
                WE = wpool.tile([P, WE10], BF16, name=f"WE_{t}")
                nc.vector.tensor_tensor(WE[:], w3[:, 0:WE10], ec[:], ALU.mult)

                D2c = wpool.tile([P, WRP], BF16, name=f"D2c_{t}")
                nc.gpsimd.tensor_tensor(D2c[:], D_c[t][:, 0:WRP],
                                        D_c[t][:, 0:WRP], ALU.mult)
                D3 = wpool.tile([P, WRP], BF16, name=f"D3_{t}")
                nc.gpsimd.tensor_tensor(D3[:], D2c[:], D_c[t][:, 0:WRP],
                                        ALU.mult)
                repl = wpool.tile([P, WRP], BF16, name=f"repl_{t}")
                nc.scalar.activation(repl[:], D3[:], AF.Exp, scale=-0.3,
                                     bias=sc(t, 7))

                # ---- stage 1: per-i partials (maps as stationary lhsT)
                # vals: elec 6/block (0:96), vdw 2/block: WS 4 blocks
                # (96:104), WE 3 blocks (104:110), repl 2 blocks (110:114)
                vals = pvals.tile([P, 120], F32, name=f"vals_{t}",
                                  tag="vals")
                for b in range(NB):
                    bl = slice(b * P, (b + 1) * P)
                    nc.tensor.matmul(vals[:, 6 * b:6 * b + 4],
                                     invD[t][:, bl], t_wts[:, 8 * t:8 * t + 4],
                                     start=True, stop=True)
                    nc.tensor.matmul(vals[:, 6 * b + 4:6 * b + 6],
                                     invD2[t][:, bl],
                                     t_wts[:, 8 * t + 2:8 * t + 4],
                                     start=True, stop=True)
                for b in range(4):
                    nc.tensor.matmul(vals[:, 96 + 2 * b:96 + 2 * b + 2],
                                     WS[:, b * P:(b + 1) * P],
                                     t_wts[:, 8 * t + 4:8 * t + 6],
                                     start=True, stop=True)
                for b in range(3):
                    nc.tensor.matmul(vals[:, 104 + 2 * b:104 + 2 * b + 2],
                                     WB[:, b * P:(b + 1) * P],
                                     t_wts[:, 8 * t + 4:8 * t + 6],
                                     start=True, stop=True)
                for b in range(3):
                    nc.tensor.matmul(vals[:, 110 + 2 * b:110 + 2 * b + 2],
                                     WE[:, b * P:(b + 1) * P],
                                     t_wts[:, 8 * t + 4:8 * t + 6],
                                     start=True, stop=True)
                for b in range(2):
                    nc.tensor.matmul(vals[:, 116 + 2 * b:116 + 2 * b + 2],
                                     repl[:, b * P:(b + 1) * P],
                                     t_wts[:, 8 * t + 6:8 * t + 8],
                                     start=True, stop=True)

                svals = wpool.tile([P, 120], F32, name=f"sv_{t}")
                nc.vector.tensor_scalar(svals[:, 0:96], vals[:, 0:96],
                                        1.0, None, ALU.mult)
                nc.vector.tensor_scalar(svals[:, 96:120], vals[:, 96:120],
                                        1.0, None, ALU.mult)

                # ---- stage 2: contract over i with per-atom weights
                for b in range(NB):
                    nc.tensor.matmul(t_out_e[32 * t:32 * t + 6, 0:6],
                                     svals[:, 6 * b:6 * b + 6],
                                     t_wq[:, 6 * NB * t + 6 * b:
                                          6 * NB * t + 6 * b + 6],
                                     start=(b == 0), stop=(b == NB - 1))
                v2 = ([(96 + 2 * b, b) for b in range(4)]
                      + [(104 + 2 * b, b) for b in range(3)]
                      + [(110 + 2 * b, b) for b in range(3)]
                      + [(116 + 2 * b, b) for b in range(2)])
                for k, (col, b) in enumerate(v2):
                    nc.tensor.matmul(t_out_v[64:66, 6 + 6 * t:12 + 6 * t],
                                     svals[:, col:col + 2],
                                     t_wq[:, 6 * NB * t + 6 * b:
                                          6 * NB * t + 6 * b + 6],
                                     start=(k == 0), stop=(k == len(v2) - 1))

            # ---- evacuate the cell groups (partition-aligned copies into
            # one zeroed SBUF tile, then a single DMA)
            sb_out = wpool.tile([66, 18], F32, name="sb_out")
            nc.gpsimd.memset(sb_out[:], 0.0)
            for t in range(JT):
                nc.vector.tensor_scalar(sb_out[32 * t:32 * t + 6, 0:6],
                                        t_out[32 * t:32 * t + 6, 0:6],
                                        1.0, None, ALU.mult)
            nc.vector.tensor_scalar(sb_out[64:66, :],
                                    t_out[64:66, :],
                                    1.0, None, ALU.mult)
            nc.sync.dma_start(cells_out[:], sb_out[:])

    _split_excess_waits(nc)
    _CACHE["nc"] = nc
    return nc


# --------------------------------------------------------------- host side
def _kd_order(X):
    out = []

    def rec(ids):
        if len(ids) <= P:
            out.append(ids)
            return
        spans = X[ids].max(0) - X[ids].min(0)
        ax = int(np.argmax(spans))
        order = ids[np.argsort(X[ids, ax], kind="stable")]
        half = (len(ids) // 2 // P) * P
        rec(order[:half])
        rec(order[half:])

    rec(np.arange(len(X)))
    return np.concatenate(out)


def _f16_split(x):
    h = x.astype(np.float16)
    l = (x - h.astype(np.float64)).astype(np.float16)
    return h, l


def _host_pre(inputs):
    f32, f64 = np.float32, np.float64
    X = np.asarray(inputs["X"], f32)
    embs = np.asarray(inputs["embs"], f32)
    qs = np.asarray(inputs["qs"], f32)
    w0 = np.asarray(inputs["w0"], f32)
    s0 = np.asarray(inputs["s0"], f32)
    c = np.asarray(inputs["chainidx"]).astype(f32)
    f = np.asarray(inputs["sf_elec"], f32)[:, 0]
    rf = np.asarray(inputs["radius_factor"], f32)[:, 0]
    df = np.asarray(inputs["depth_factor"], f32)[:, 0]

    X64 = X.astype(f64)
    Xc64 = X64 - X64.mean(0)
    r2_64 = (Xc64 ** 2).sum(1)
    D2x = r2_64[:, None] + r2_64[None, :] - 2.0 * (Xc64 @ Xc64.T)
    np.fill_diagonal(D2x, 0.0)
    D2x = np.maximum(D2x, 0.0)

    perm = _kd_order(Xc64)

    # sorted-frame quantities
    r2s = r2_64[perm]
    D2s = D2x[perm][:, perm]
    Xs = Xc64[perm]
    sfa = (embs @ f[:C]).astype(f64)[perm]
    sfb = (embs @ f[C:2 * C]).astype(f64)[perm]
    f16 = float(f[2 * C])
    ar = (embs @ rf[:C]).astype(f64)[perm]
    br = (embs @ rf[C:]).astype(f64)[perm]
    ad = (embs @ df[:C]).astype(f64)[perm]
    bd = (embs @ df[C:]).astype(f64)[perm]
    w0j = np.sqrt(w0.astype(f64) ** 2 + 1e-6)[perm]
    qs_s = qs.astype(f64)[perm]
    c_s = c.astype(f64)[perm]
    s0_s = s0.astype(f64)[perm]

    hx, lx = _f16_split(Xs)
    hr2j, lr2j = _f16_split(r2s)
    hr2i, lr2i = _f16_split(r2s + 3e-6)

    pkid_m = (np.eye(P, dtype=f32) * POKE).astype(ml_dtypes.bfloat16)
    u3 = qs_s * c_s
    u4 = qs_s * (1.0 - 2.0 * c_s)

    in_maps = []
    for core in range(NCORES):
        m = {}
        pks_m = np.zeros((P, P + 2 * WC), f32)
        pks_m[:, 0:P] = np.eye(P, dtype=f32) * POKE
        scal_m = np.zeros((P, 8 * JT), f32)
        wts_m = np.zeros((P, 8 * JT), f32)
        wq_m = np.zeros((P, 6 * NB * JT), f32)
        for t in range(JT):
            g0 = core * (P * JT) + t * P
            jj = slice(g0, g0 + P)
            minD2 = D2s[jj].min(0)
            pi = np.argsort(minD2, kind="stable")   # full 2048 permutation

            geo_m = np.zeros((13, N + P + P + 2 * WEA), np.float16)
            geo_r = geo_m[:, 0:N]
            geoT_m = geo_m[:, N:N + P]
            geo_m[0, N + P:N + 2 * P] = 1.0
            geo_m[1, N + P:N + 2 * P] = ar[jj].astype(np.float16)
            geo_m[2, N + P:N + 2 * P] = ad[jj].astype(np.float16)
            geo_m[0, N + 2 * P:N + 2 * P + WEA] = br[pi[:WEA]].astype(
                np.float16)
            geo_m[0, N + 2 * P + WEA:] = bd[pi[:WEA]].astype(np.float16)
            geo_m[1, N + 2 * P:N + 2 * P + WEA] = 1.0
            geo_m[2, N + 2 * P + WEA:] = 1.0
            for d in range(3):
                geo_r[3 * d + 0] = hx[pi, d]
                geo_r[3 * d + 1] = lx[pi, d]
                geo_r[3 * d + 2] = hx[pi, d]
                geoT_m[3 * d + 0] = -2.0 * hx[jj, d]
                geoT_m[3 * d + 1] = -2.0 * hx[jj, d]
                geoT_m[3 * d + 2] = -2.0 * lx[jj, d]
            geo_r[9] = 1.0
            geo_r[10] = 1.0
            geoT_m[9] = hr2j[jj]
            geoT_m[10] = lr2j[jj]
            geo_r[11] = hr2i[pi]
            geo_r[12] = lr2i[pi]
            geoT_m[11] = 1.0
            geoT_m[12] = 1.0

            pos = np.empty(N, np.int64)
            pos[pi] = np.arange(N)
            pk_m = np.zeros((P, WC), f32)
            pk_m[np.arange(P), pos[g0 + np.arange(P)]] = POKE
            nj, ni_ = np.where(D2s[jj] < NEAR_TH2)
            sel = ni_ != (g0 + nj)
            pk_m[nj[sel], pos[ni_[sel]]] = POKE

            m[f"geo{t}"] = geo_m
            pks_m[:, P + t * WC:P + (t + 1) * WC] = pk_m / POKE

            scal_m[:, 8 * t + 0] = ar[jj]
            scal_m[:, 8 * t + 1] = ad[jj]
            scal_m[:, 8 * t + 2] = 1.6 * s0_s[jj]
            scal_m[:, 8 * t + 3] = 0.8 * s0_s[jj]
            scal_m[:, 8 * t + 4] = w0j[jj] * (SQPI / 6.0)
            scal_m[:, 8 * t + 5] = w0j[jj] * (SQPI / 12.0)
            scal_m[:, 8 * t + 6] = -0.3
            scal_m[:, 8 * t + 7] = LN5
            wts_m[:, 8 * t + 0] = u3[jj] * sfa[jj]
            wts_m[:, 8 * t + 1] = u4[jj] * sfa[jj]
            wts_m[:, 8 * t + 2] = u3[jj]
            wts_m[:, 8 * t + 3] = u4[jj]
            wts_m[:, 8 * t + 4] = c_s[jj]
            wts_m[:, 8 * t + 5] = 1.0 - 2.0 * c_s[jj]
            wts_m[:, 8 * t + 6] = -c_s[jj]
            wts_m[:, 8 * t + 7] = -(1.0 - 2.0 * c_s[jj])
            for b in range(NB):
                ib = pi[b * P:(b + 1) * P]
                base = 6 * NB * t + 6 * b
                wq_m[:, base + 0] = qs_s[ib]
                wq_m[:, base + 1] = qs_s[ib] * c_s[ib]
                wq_m[:, base + 2] = qs_s[ib] * sfb[ib]
                wq_m[:, base + 3] = qs_s[ib] * sfb[ib] * c_s[ib]
                wq_m[:, base + 4] = 1.0
                wq_m[:, base + 5] = c_s[ib]
        m["pks"] = pks_m.astype(ml_dtypes.bfloat16)
        m["scal"] = scal_m
        m["scw"] = wq_m
        m["wts"] = wts_m.astype(np.float16)
        in_maps.append(m)

    # ---- exact fp64 contributions of the poked near pairs (device ~0)
    ni_a, nj_a = np.where((D2s < NEAR_TH2) & (D2s > 0))
    e_elec_corr = 0.0
    e_vdw_corr = 0.0
    if len(ni_a):
        msk = c_s[ni_a] != c_s[nj_a]
        ia, ja = ni_a[msk], nj_a[msk]       # ordered pairs, both directions
        Dn = np.sqrt(D2s[ia, ja] + 3e-6)
        invDn = 1.0 / (Dn + 1e-6)
        e_elec_corr = 0.5 * CONV * np.sum(
            qs_s[ia] * qs_s[ja] * invDn
            * (sfa[ja] + sfb[ia] + f16 * invDn))
        sig_r = 1.0 / (1.0 + np.exp(-(ar[ja] + br[ia])))
        s = 2.0 * s0_s[ja] * (0.8 * sig_r + 0.4)
        repl = 5.0 * np.exp(-0.3 * Dn ** 3)
        Dmn = Dn - s
        attr = (np.exp(-(Dmn - 0.3) ** 2) + np.exp(-3.0 * Dmn * Dmn)
                + np.exp(-10.0 * Dmn * Dmn)) / 3.0
        sig_d = 1.0 / (1.0 + np.exp(-(ad[ja] + bd[ia])))
        w = w0j[ja] * (sig_d + 0.5)
        e_vdw_corr = np.sum(-w * attr + repl)

    aux = dict(inputs=inputs, f16=f16,
               e_elec_corr=e_elec_corr, e_vdw_corr=e_vdw_corr)
    return in_maps, aux


def _host_post(core_cells, aux):
    f64 = np.float64
    f16 = aux["f16"]
    E_elec = 0.0
    E_vdw = 0.0
    for cells in core_cells:
        cc = cells.astype(f64)
        for t in range(JT):
            e = cc[32 * t:32 * t + 6, 0:6]
            v = cc[64:66, 6 + 6 * t:12 + 6 * t]
            E_elec += (e[0, 0] + e[1, 1] + e[2, 2] + e[3, 3]
                       + f16 * (e[4, 0] + e[5, 1]))
            E_vdw += -(v[0, 4] + v[1, 5])
    E_elec = 0.5 * CONV * E_elec + aux["e_elec_corr"]
    E_vdw = E_vdw + aux["e_vdw_corr"]

    inputs = aux["inputs"]
    embs = np.asarray(inputs["embs"], np.float32)
    die = np.asarray(inputs["die_factor"], np.float32)
    born = np.asarray(inputs["born_factor"], np.float32)
    qsf = np.asarray(inputs["qs"], np.float32).astype(f64)
    atomic_die = (embs @ die + 1e-6).astype(f64)
    R = (embs @ born + 1.0).astype(f64)
    E_self = -(1.0 - 1.0 / atomic_die) * qsf / (R + 1e-6)
    E_solv = CONV * np.sum(E_self) * 0.01

    def guard(e):
        return np.float32(1e-6) if np.isnan(e) else np.float32(e)

    return np.asarray([guard(E_vdw), guard(E_elec), guard(E_solv)],
                      dtype=np.float32)


def kernel(**inputs):
    nc = _build()
    in_maps, aux = _host_pre(inputs)
    res = run_bass_kernel_spmd(nc, in_maps, list(range(NCORES)))
    core_cells = [res.results[cid]["cells"] for cid in range(NCORES)]
    return _host_post(core_cells, aux)


if __name__ == "__main__":
    pass


# revision 27
# speedup vs baseline: 2.9776x; 1.0079x over previous
"""EnergyNet Trainium2 kernel v3 (SPMD over 8 NeuronCores).

Layout: partitions = j (each core owns 256 j's as 2 tiles of 128), free
dim = i. Each tile gets its OWN permutation of the i axis: columns
sorted by min-distance to the tile's 128 atoms (atoms are k-d ordered so
a 128-block's neighborhood is compact), so the 512-column prefix holds
every pair within the vdW cutoff. Electrostatics run full width; the
vdW chain (sigmoids + 3 Gaussians via Derivative_Erf + repulsion exp)
runs on shrinking prefixes (448/384/320/256).

D^2 comes from one k=13 fp16 hi/lo-split Gram matmul (PE multiplies
fp16 exactly, PSUM accumulates fp32; |err| ~ 5e-4). Near pairs
(D^2 < 0.25) and the diagonal get +1e6 pokes so their device
contribution is ~0 (elec) / exactly 0 (vdW); the host adds their exact
fp64 contributions.

Both reduction stages run on the PE: stage 1 uses the maps (invD,
invD2, vdw) as stationary lhsT against per-j weight columns, giving
per-i partials; stage 2 contracts those over i with per-atom weight
columns (qs, qs*c, qs*sfb, qs*sfb*c, 1, c), accumulating 6x6 / 2x6
energy cells in PSUM. The host combines 2*16*8 fp32 cells in fp64.
"""
import numpy as np
import ml_dtypes

import concourse.bass as bass
import concourse.mybir as mybir
import bass_rust as _bass_rust
from concourse.bass_utils import run_bass_kernel_spmd
from concourse.tile import TileContext

N = 2048
C = 8
CONV = 332.07156
NCORES = 8
P = 128
JT = 2          # j-tiles per core
WC = 512        # compact prefix (pokes + vdW support)
WEA = 448       # width for exp(-(Dm-0.3)^2) and the vdw map
WE3 = 384       # width for exp(-3 Dm^2)
WE10 = 384      # width for exp(-10 Dm^2); 3 full 128-blocks
WRP = 256       # width for repulsion 5 exp(-0.3 D^3)
CUT = 9.0       # neighbor cutoff (A) for the compact prefix
NEAR_TH2 = 0.25
POKE = 1.0e6
NB = 16         # stage-1 i-blocks of 128
LN5 = float(np.log(5.0))
SQ3 = float(np.sqrt(3.0))
SQ10 = float(np.sqrt(10.0))
SQPI = float(np.sqrt(np.pi))

AF = mybir.ActivationFunctionType
ALU = mybir.AluOpType
F32 = mybir.dt.float32
BF16 = mybir.dt.bfloat16
FP16 = mybir.dt.float16


# --------------------------------------------------------------- patches
def _patched_drain_and_barrier(self, tick_clock, wait_clock):
    gc = tick_clock.global_clock
    try:
        n_procs = len(gc)
    except TypeError:
        n_procs = 27
    ticks = [gc[p] for p in range(n_procs)]
    for p in [p for p in range(n_procs) if ticks[p] > 0] or [0]:
        d = self.nc.sync.drain()
        sub = [ticks[q] if q == p else 0 for q in range(n_procs)]
        wait_clock.add_sem_waits(
            d.ins, _bass_rust.ScopedClock({None: _bass_rust.VectorClock(sub)})
        )
    self.nc.all_engine_barrier()
    assert self.sems is not None
    popped = self.nc._tile_sem_poison_stack.pop()
    assert popped is self._sem_poison
    self.nc.clear_and_free_semaphores(list(self.sems.allocated().values()))


TileContext._drain_and_barrier = _patched_drain_and_barrier

_NOPC = [0]


def _split_excess_waits(nc):
    """This walrus build rejects instructions carrying more than one sem
    wait. Hoist excess waits onto same-engine NoOps inserted just before
    the offending instruction (the engine sequencer executes them in
    order, so the waits still gate it)."""
    for blk in nc.m.functions[0].blocks:
        insts = blk.instructions
        out = []
        changed = False
        for inst in insts:
            si = inst.sync_info
            waits = list(si.on_wait) if si is not None else []
            if len(waits) > 1:
                keep_idx = len(waits) - 1
                if type(inst).__name__ == "InstDMACopy":
                    for k, w in enumerate(waits):
                        if str(getattr(w, "ant_name", "")).startswith(
                                ("DMAHW", "DMASW")):
                            keep_idx = k
                            break
                rest = [w for k, w in enumerate(waits) if k != keep_idx]
                for w in rest:
                    _NOPC[0] += 1
                    nop = mybir.InstNoOp(name=f"WH-{_NOPC[0]}", ins=[], outs=[])
                    nop.engine = inst.engine
                    nop.sync_info = mybir.SyncInfo(on_wait=[w], on_update=[])
                    out.append(nop)
                inst.sync_info = mybir.SyncInfo(on_wait=[waits[keep_idx]],
                                                on_update=list(si.on_update))
                changed = True
            out.append(inst)
        if changed:
            blk.instructions = out


def _bcast_src(dram_ap, n_free):
    """Stride-0 partition AP: read one DRAM row into all 128 partitions."""
    return bass.AP(tensor=dram_ap.tensor, offset=0,
                   ap=_bass_rust.VecI64Pair([[0, P], [1, n_free]]))


_CACHE = {}


def _build():
    if "nc" in _CACHE:
        return _CACHE["nc"]
    nc = bass.Bass()
    # geo cols: [0:N rhs | N:N+P lhsT | +P one-hot lhsT | +2*WEA br/bd packs]
    GEOW = N + P + P + 2 * WEA
    geo = [nc.declare_dram_parameter(f"geo{t}", [13, GEOW], FP16,
                                     isOutput=False) for t in range(JT)]
    pks = nc.declare_dram_parameter("pks", [P, P + 2 * WC], BF16,
                                    isOutput=False)
    scal_d = nc.declare_dram_parameter("scal", [P, 8 * JT], F32,
                                       isOutput=False)
    scw = nc.declare_dram_parameter("scw", [P, 6 * NB * JT], F32,
                                    isOutput=False)
    wts = nc.declare_dram_parameter("wts", [P, 8 * JT], FP16, isOutput=False)
    cells_out = nc.declare_dram_parameter("cells", [66, 18], F32,
                                          isOutput=True)

    with TileContext(nc) as tc:
        with tc.tile_pool(name="const", bufs=1) as cpool, \
             tc.tile_pool(name="work", bufs=1) as wpool, \
             tc.tile_pool(name="gpin", bufs=2, space="PSUM") as gpin, \
             tc.tile_pool(name="gring", bufs=2, space="PSUM") as gring, \
             tc.tile_pool(name="pvals", bufs=1, space="PSUM") as pvals, \
             tc.tile_pool(name="pbc", bufs=1, space="PSUM") as pbc, \
             tc.tile_pool(name="pout", bufs=1, space="PSUM") as poutp:

            t_geofull = [cpool.tile([13, N + P + P + 2 * WEA], FP16,
                                    name=f"t_geo{t}") for t in range(JT)]
            t_geo = [g[:, 0:N] for g in t_geofull]
            t_geoT = [g[:, N:N + P] for g in t_geofull]
            t_bone = [g[:, N + P:N + 2 * P] for g in t_geofull]
            t_bpack = [g[:, N + 2 * P:] for g in t_geofull]
            t_pks = cpool.tile([P, P + 2 * WC], BF16, name="t_pks")
            t_pkid = t_pks[:, 0:P]
            t_pk = [t_pks[:, P + t * WC:P + (t + 1) * WC] for t in range(JT)]
            t_scal_t = cpool.tile([P, 8 * JT], F32, name="t_scal")
            t_scal = t_scal_t[:, :]
            t_scw = cpool.tile([P, 6 * NB * JT], F32, name="t_scw")
            t_wq = t_scw[:, :]
            t_wts = cpool.tile([P, 8 * JT], FP16, name="t_wts")
            # DMA order = need order
            nc.sync.dma_start(t_geofull[0][:], geo[0][:])
            nc.sync.dma_start(t_scal_t[:], scal_d[:])
            nc.sync.dma_start(t_pks[:], pks[:])
            nc.sync.dma_start(t_geofull[1][:], geo[1][:])
            nc.sync.dma_start(t_scw[:], scw[:])
            nc.sync.dma_start(t_wts[:], wts[:])

            # out cells (matmul col base must be 0/32/64):
            # elec t@[32t:32t+6, 0:6], vdw t@[64:66, 6+6t:12+6t]
            t_out = poutp.tile([66, 18], F32, name="t_out")
            t_out_e = t_out[:, 0:6]
            t_out_v = t_out[:, 0:18]

            def sc(t, k):
                return t_scal[:, 8 * t + k:8 * t + k + 1]

            # ---- sigmoid args via k=3 PE matmul: br_i/bd_i data row plus
            # ar_j/ad_j bias rows against segment indicators; one bias-free
            # sigmoid over both segments reads the PSUM directly
            sigr, sigd = [], []
            for t in range(JT):
                pb = pbc.tile([P, 2 * WEA], F32, name=f"bc_{t}", tag="bc")
                nc.tensor.matmul(pb[:, 0:WEA], t_bone[t][0:3, :],
                                 t_bpack[t][0:3, 0:WEA],
                                 start=True, stop=True)
                nc.tensor.matmul(pb[:, WEA:2 * WEA], t_bone[t][0:3, :],
                                 t_bpack[t][0:3, WEA:2 * WEA],
                                 start=True, stop=True)
                sg = wpool.tile([P, 2 * WEA], FP16, name=f"sigs_{t}")
                nc.scalar.activation(sg[:, 0:WEA], pb[:, 0:WEA], AF.Sigmoid)
                nc.scalar.activation(sg[:, WEA:2 * WEA], pb[:, WEA:2 * WEA],
                                     AF.Sigmoid)
                sigr.append(sg[:, 0:WEA])
                sigd.append(sg[:, WEA:2 * WEA])

            # ---- per tile: Gram -> invD2 -> invD -> D_c -> Dm
            invD, invD2, D_c, Dm = [], [], [], []
            for t in range(JT):
                Gb = []
                for cidx in range(4):
                    if cidx == 0:
                        g = gpin.tile([P, 512], F32, name=f"G_{t}_0",
                                      tag="Gpin")
                    else:
                        g = gring.tile([P, 512], F32, name=f"G_{t}_{cidx}",
                                       tag="G")
                    cs = 512 * cidx
                    nc.tensor.matmul(g[:], t_geoT[t][:],
                                     t_geo[t][0:13, cs:cs + 512],
                                     start=True, stop=(cidx != 0))
                    if cidx == 0:
                        nc.tensor.matmul(g[:], t_pkid[:], t_pk[t][:],
                                         start=False, stop=True)
                    Gb.append(g)

                iD2 = wpool.tile([P, N], FP16, name=f"invD2_{t}")
                with nc.allow_low_precision(reason="fp16 maps; reductions "
                                            "accumulate fp32 in PSUM"):
                    for cidx in range(4):
                        sl = slice(cidx * 512, (cidx + 1) * 512)
                        nc.vector.reciprocal(iD2[:, sl], Gb[cidx][:])
                iD = wpool.tile([P, N], FP16, name=f"invD_{t}")
                for h in range(2):
                    sl = slice(h * 1024, (h + 1) * 1024)
                    nc.scalar.activation(iD[:, sl], iD2[:, sl], AF.Sqrt)
                invD2.append(iD2)
                invD.append(iD)

                # D_c needs only invD[:, 0:WEA] (first sqrt half) + G0
                dc = wpool.tile([P, WEA], FP16, name=f"Dc_{t}")
                nc.vector.tensor_tensor(dc[:], Gb[0][:, 0:WEA],
                                        iD[:, 0:WEA], ALU.mult)
                s_m = wpool.tile([P, WEA], FP16, name=f"s_{t}")
                nc.vector.tensor_scalar(s_m[:], sigr[t][:], sc(t, 2),
                                        sc(t, 3), ALU.mult, ALU.add)
                dm = wpool.tile([P, WEA], FP16, name=f"Dm_{t}")
                nc.vector.tensor_tensor(dm[:], dc[:], s_m[:], ALU.subtract)
                D_c.append(dc)
                Dm.append(dm)

            # ---- per tile: Gaussians + repulsion + vdw map + reductions
            for t in range(JT):
                ea = wpool.tile([P, WEA], BF16, name=f"ea_{t}")
                nc.scalar.activation(ea[:], Dm[t][:], AF.Derivative_Erf,
                                     bias=sc(t, 6))
                eb = wpool.tile([P, WE3], BF16, name=f"eb_{t}")
                nc.scalar.activation(eb[:], Dm[t][:, 0:WE3],
                                     AF.Derivative_Erf, scale=SQ3)
                ec = wpool.tile([P, WE10], BF16, name=f"ec_{t}")
                nc.scalar.activation(ec[:], Dm[t][:, 0:WE10],
                                     AF.Derivative_Erf, scale=SQ10)
                w3 = wpool.tile([P, WEA], BF16, name=f"w3_{t}")
                nc.vector.tensor_scalar(w3[:], sigd[t][:], sc(t, 4), sc(t, 5),
                                        ALU.mult, ALU.add)
                WS = wpool.tile([P, WC], BF16, name=f"WS_{t}")
                nc.gpsimd.memset(WS[:, WEA:WC], 0.0)
                nc.vector.tensor_tensor(WS[:, 0:WEA], w3[:], ea[:], ALU.mult)
                WB = wpool.tile([P, WE3], BF16, name=f"WB_{t}")
                nc.vector.tensor_tensor(WB[:], w3[:, 0:WE3], eb[:], ALU.mult)
                WE = wpool.tile([P, WE10], BF16, name=f"WE_{t}")
                nc.vector.tensor_tensor(WE[:], w3[:, 0:WE10], ec[:], ALU.mult)

                D2c = wpool.tile([P, WRP], BF16, name=f"D2c_{t}")
                nc.gpsimd.tensor_tensor(D2c[:], D_c[t][:, 0:WRP],
                                        D_c[t][:, 0:WRP], ALU.mult)
                D3 = wpool.tile([P, WRP], BF16, name=f"D3_{t}")
                nc.gpsimd.tensor_tensor(D3[:], D2c[:], D_c[t][:, 0:WRP],
                                        ALU.mult)
                repl = wpool.tile([P, WRP], BF16, name=f"repl_{t}")
                nc.scalar.activation(repl[:], D3[:], AF.Exp, scale=-0.3,
                                     bias=sc(t, 7))

                # ---- stage 1: per-i partials (maps as stationary lhsT)
                # vals: elec 6/block (0:96), vdw 2/block: WS 4 blocks
                # (96:104), WE 3 blocks (104:110), repl 2 blocks (110:114)
                vals = pvals.tile([P, 120], F32, name=f"vals_{t}",
                                  tag="vals")
                for b in range(NB):
                    bl = slice(b * P, (b + 1) * P)
                    nc.tensor.matmul(vals[:, 6 * b:6 * b + 4],
                                     invD[t][:, bl], t_wts[:, 8 * t:8 * t + 4],
                                     start=True, stop=True)
                    nc.tensor.matmul(vals[:, 6 * b + 4:6 * b + 6],
                                     invD2[t][:, bl],
                                     t_wts[:, 8 * t + 2:8 * t + 4],
                                     start=True, stop=True)
                for b in range(4):
                    nc.tensor.matmul(vals[:, 96 + 2 * b:96 + 2 * b + 2],
                                     WS[:, b * P:(b + 1) * P],
                                     t_wts[:, 8 * t + 4:8 * t + 6],
                                     start=True, stop=True)
                for b in range(3):
                    nc.tensor.matmul(vals[:, 104 + 2 * b:104 + 2 * b + 2],
                                     WB[:, b * P:(b + 1) * P],
                                     t_wts[:, 8 * t + 4:8 * t + 6],
                                     start=True, stop=True)
                for b in range(3):
                    nc.tensor.matmul(vals[:, 110 + 2 * b:110 + 2 * b + 2],
                                     WE[:, b * P:(b + 1) * P],
                                     t_wts[:, 8 * t + 4:8 * t + 6],
                                     start=True, stop=True)
                for b in range(2):
                    nc.tensor.matmul(vals[:, 116 + 2 * b:116 + 2 * b + 2],
                                     repl[:, b * P:(b + 1) * P],
                                     t_wts[:, 8 * t + 6:8 * t + 8],
                                     start=True, stop=True)

                svals = wpool.tile([P, 120], F32, name=f"sv_{t}")
                nc.vector.tensor_scalar(svals[:, 0:96], vals[:, 0:96],
                                        1.0, None, ALU.mult)
                nc.vector.tensor_scalar(svals[:, 96:120], vals[:, 96:120],
                                        1.0, None, ALU.mult)

                # ---- stage 2: contract over i with per-atom weights
                for b in range(NB):
                    nc.tensor.matmul(t_out_e[32 * t:32 * t + 6, 0:6],
                                     svals[:, 6 * b:6 * b + 6],
                                     t_wq[:, 6 * NB * t + 6 * b:
                                          6 * NB * t + 6 * b + 6],
                                     start=(b == 0), stop=(b == NB - 1))
                v2 = ([(96 + 2 * b, b) for b in range(4)]
                      + [(104 + 2 * b, b) for b in range(3)]
                      + [(110 + 2 * b, b) for b in range(3)]
                      + [(116 + 2 * b, b) for b in range(2)])
                for k, (col, b) in enumerate(v2):
                    nc.tensor.matmul(t_out_v[64:66, 6 + 6 * t:12 + 6 * t],
                                     svals[:, col:col + 2],
                                     t_wq[:, 6 * NB * t + 6 * b:
                                          6 * NB * t + 6 * b + 6],
                                     start=(k == 0), stop=(k == len(v2) - 1))

            # ---- evacuate the cell groups (partition-aligned copies into
            # one zeroed SBUF tile, then a single DMA)
            sb_out = wpool.tile([66, 18], F32, name="sb_out")
            nc.gpsimd.memset(sb_out[:], 0.0)
            for t in range(JT):
                nc.vector.tensor_scalar(sb_out[32 * t:32 * t + 6, 0:6],
                                        t_out[32 * t:32 * t + 6, 0:6],
                                        1.0, None, ALU.mult)
            nc.vector.tensor_scalar(sb_out[64:66, :],
                                    t_out[64:66, :],
                                    1.0, None, ALU.mult)
            nc.sync.dma_start(cells_out[:], sb_out[:])

    _split_excess_waits(nc)
    _CACHE["nc"] = nc
    return nc


# --------------------------------------------------------------- host side
def _kd_order(X):
    out = []

    def rec(ids):
        if len(ids) <= P:
            out.append(ids)
            return
        spans = X[ids].max(0) - X[ids].min(0)
        ax = int(np.argmax(spans))
        order = ids[np.argsort(X[ids, ax], kind="stable")]
        half = (len(ids) // 2 // P) * P
        rec(order[:half])
        rec(order[half:])

    rec(np.arange(len(X)))
    return np.concatenate(out)


def _f16_split(x):
    h = x.astype(np.float16)
    l = (x - h.astype(np.float64)).astype(np.float16)
    return h, l


def _host_pre(inputs):
    f32, f64 = np.float32, np.float64
    X = np.asarray(inputs["X"], f32)
    embs = np.asarray(inputs["embs"], f32)
    qs = np.asarray(inputs["qs"], f32)
    w0 = np.asarray(inputs["w0"], f32)
    s0 = np.asarray(inputs["s0"], f32)
    c = np.asarray(inputs["chainidx"]).astype(f32)
    f = np.asarray(inputs["sf_elec"], f32)[:, 0]
    rf = np.asarray(inputs["radius_factor"], f32)[:, 0]
    df = np.asarray(inputs["depth_factor"], f32)[:, 0]

    X64 = X.astype(f64)
    Xc64 = X64 - X64.mean(0)
    r2_64 = (Xc64 ** 2).sum(1)
    D2x = r2_64[:, None] + r2_64[None, :] - 2.0 * (Xc64 @ Xc64.T)
    np.fill_diagonal(D2x, 0.0)
    D2x = np.maximum(D2x, 0.0)

    perm = _kd_order(Xc64)

    # sorted-frame quantities
    r2s = r2_64[perm]
    D2s = D2x[perm][:, perm]
    Xs = Xc64[perm]
    sfa = (embs @ f[:C]).astype(f64)[perm]
    sfb = (embs @ f[C:2 * C]).astype(f64)[perm]
    f16 = float(f[2 * C])
    ar = (embs @ rf[:C]).astype(f64)[perm]
    br = (embs @ rf[C:]).astype(f64)[perm]
    ad = (embs @ df[:C]).astype(f64)[perm]
    bd = (embs @ df[C:]).astype(f64)[perm]
    w0j = np.sqrt(w0.astype(f64) ** 2 + 1e-6)[perm]
    qs_s = qs.astype(f64)[perm]
    c_s = c.astype(f64)[perm]
    s0_s = s0.astype(f64)[perm]

    hx, lx = _f16_split(Xs)
    hr2j, lr2j = _f16_split(r2s)
    hr2i, lr2i = _f16_split(r2s + 3e-6)

    pkid_m = (np.eye(P, dtype=f32) * POKE).astype(ml_dtypes.bfloat16)
    u3 = qs_s * c_s
    u4 = qs_s * (1.0 - 2.0 * c_s)

    in_maps = []
    for core in range(NCORES):
        m = {}
        pks_m = np.zeros((P, P + 2 * WC), f32)
        pks_m[:, 0:P] = np.eye(P, dtype=f32) * POKE
        scal_m = np.zeros((P, 8 * JT), f32)
        wts_m = np.zeros((P, 8 * JT), f32)
        wq_m = np.zeros((P, 6 * NB * JT), f32)
        for t in range(JT):
            g0 = core * (P * JT) + t * P
            jj = slice(g0, g0 + P)
            minD2 = D2s[jj].min(0)
            pi = np.argsort(minD2, kind="stable")   # full 2048 permutation

            geo_m = np.zeros((13, N + P + P + 2 * WEA), np.float16)
            geo_r = geo_m[:, 0:N]
            geoT_m = geo_m[:, N:N + P]
            geo_m[0, N + P:N + 2 * P] = 1.0
            geo_m[1, N + P:N + 2 * P] = ar[jj].astype(np.float16)
            geo_m[2, N + P:N + 2 * P] = ad[jj].astype(np.float16)
            geo_m[0, N + 2 * P:N + 2 * P + WEA] = br[pi[:WEA]].astype(
                np.float16)
            geo_m[0, N + 2 * P + WEA:] = bd[pi[:WEA]].astype(np.float16)
            geo_m[1, N + 2 * P:N + 2 * P + WEA] = 1.0
            geo_m[2, N + 2 * P + WEA:] = 1.0
            for d in range(3):
                geo_r[3 * d + 0] = hx[pi, d]
                geo_r[3 * d + 1] = lx[pi, d]
                geo_r[3 * d + 2] = hx[pi, d]
                geoT_m[3 * d + 0] = -2.0 * hx[jj, d]
                geoT_m[3 * d + 1] = -2.0 * hx[jj, d]
                geoT_m[3 * d + 2] = -2.0 * lx[jj, d]
            geo_r[9] = 1.0
            geo_r[10] = 1.0
            geoT_m[9] = hr2j[jj]
            geoT_m[10] = lr2j[jj]
            geo_r[11] = hr2i[pi]
            geo_r[12] = lr2i[pi]
            geoT_m[11] = 1.0
            geoT_m[12] = 1.0

            pos = np.empty(N, np.int64)
            pos[pi] = np.arange(N)
            pk_m = np.zeros((P, WC), f32)
            pk_m[np.arange(P), pos[g0 + np.arange(P)]] = POKE
            nj, ni_ = np.where(D2s[jj] < NEAR_TH2)
            sel = ni_ != (g0 + nj)
            pk_m[nj[sel], pos[ni_[sel]]] = POKE

            m[f"geo{t}"] = geo_m
            pks_m[:, P + t * WC:P + (t + 1) * WC] = pk_m / POKE

            scal_m[:, 8 * t + 0] = ar[jj]
            scal_m[:, 8 * t + 1] = ad[jj]
            scal_m[:, 8 * t + 2] = 1.6 * s0_s[jj]
            scal_m[:, 8 * t + 3] = 0.8 * s0_s[jj]
            scal_m[:, 8 * t + 4] = w0j[jj] * (SQPI / 6.0)
            scal_m[:, 8 * t + 5] = w0j[jj] * (SQPI / 12.0)
            scal_m[:, 8 * t + 6] = -0.3
            scal_m[:, 8 * t + 7] = LN5
            wts_m[:, 8 * t + 0] = u3[jj] * sfa[jj]
            wts_m[:, 8 * t + 1] = u4[jj] * sfa[jj]
            wts_m[:, 8 * t + 2] = u3[jj]
            wts_m[:, 8 * t + 3] = u4[jj]
            wts_m[:, 8 * t + 4] = c_s[jj]
            wts_m[:, 8 * t + 5] = 1.0 - 2.0 * c_s[jj]
            wts_m[:, 8 * t + 6] = -c_s[jj]
            wts_m[:, 8 * t + 7] = -(1.0 - 2.0 * c_s[jj])
            for b in range(NB):
                ib = pi[b * P:(b + 1) * P]
                base = 6 * NB * t + 6 * b
                wq_m[:, base + 0] = qs_s[ib]
                wq_m[:, base + 1] = qs_s[ib] * c_s[ib]
                wq_m[:, base + 2] = qs_s[ib] * sfb[ib]
                wq_m[:, base + 3] = qs_s[ib] * sfb[ib] * c_s[ib]
                wq_m[:, base + 4] = 1.0
                wq_m[:, base + 5] = c_s[ib]
        m["pks"] = pks_m.astype(ml_dtypes.bfloat16)
        m["scal"] = scal_m
        m["scw"] = wq_m
        m["wts"] = wts_m.astype(np.float16)
        in_maps.append(m)

    # ---- exact fp64 contributions of the poked near pairs (device ~0)
    ni_a, nj_a = np.where((D2s < NEAR_TH2) & (D2s > 0))
    e_elec_corr = 0.0
    e_vdw_corr = 0.0
    if len(ni_a):
        msk = c_s[ni_a] != c_s[nj_a]
        ia, ja = ni_a[msk], nj_a[msk]       # ordered pairs, both directions
        Dn = np.sqrt(D2s[ia, ja] + 3e-6)
        invDn = 1.0 / (Dn + 1e-6)
        e_elec_corr = 0.5 * CONV * np.sum(
            qs_s[ia] * qs_s[ja] * invDn
            * (sfa[ja] + sfb[ia] + f16 * invDn))
        sig_r = 1.0 / (1.0 + np.exp(-(ar[ja] + br[ia])))
        s = 2.0 * s0_s[ja] * (0.8 * sig_r + 0.4)
        repl = 5.0 * np.exp(-0.3 * Dn ** 3)
        Dmn = Dn - s
        attr = (np.exp(-(Dmn - 0.3) ** 2) + np.exp(-3.0 * Dmn * Dmn)
                + np.exp(-10.0 * Dmn * Dmn)) / 3.0
        sig_d = 1.0 / (1.0 + np.exp(-(ad[ja] + bd[ia])))
        w = w0j[ja] * (sig_d + 0.5)
        e_vdw_corr = np.sum(-w * attr + repl)

    aux = dict(inputs=inputs, f16=f16,
               e_elec_corr=e_elec_corr, e_vdw_corr=e_vdw_corr)
    return in_maps, aux


def _host_post(core_cells, aux):
    f64 = np.float64
    f16 = aux["f16"]
    E_elec = 0.0
    E_vdw = 0.0
    for cells in core_cells:
        cc = cells.astype(f64)
        for t in range(JT):
            e = cc[32 * t:32 * t + 6, 0:6]
            v = cc[64:66, 6 + 6 * t:12 + 6 * t]
            E_elec += (e[0, 0] + e[1, 1] + e[2, 2] + e[3, 3]
                       + f16 * (e[4, 0] + e[5, 1]))
            E_vdw += -(v[0, 4] + v[1, 5])
    E_elec = 0.5 * CONV * E_elec + aux["e_elec_corr"]
    E_vdw = E_vdw + aux["e_vdw_corr"]

    inputs = aux["inputs"]
    embs = np.asarray(inputs["embs"], np.float32)
    die = np.asarray(inputs["die_factor"], np.float32)
    born = np.asarray(inputs["born_factor"], np.float32)
    qsf = np.asarray(inputs["qs"], np.float32).astype(f64)
    atomic_die = (embs @ die + 1e-6).astype(f64)
    R = (embs @ born + 1.0).astype(f64)
    E_self = -(1.0 - 1.0 / atomic_die) * qsf / (R + 1e-6)
    E_solv = CONV * np.sum(E_self) * 0.01

    def guard(e):
        return np.float32(1e-6) if np.isnan(e) else np.float32(e)

    return np.asarray([guard(E_vdw), guard(E_elec), guard(E_solv)],
                      dtype=np.float32)


def kernel(**inputs):
    nc = _build()
    in_maps, aux = _host_pre(inputs)
    res = run_bass_kernel_spmd(nc, in_maps, list(range(NCORES)))
    core_cells = [res.results[cid]["cells"] for cid in range(NCORES)]
    return _host_post(core_cells, aux)


if __name__ == "__main__":
    pass


# revision 28
# speedup vs baseline: 3.0197x; 1.0142x over previous
"""EnergyNet Trainium2 kernel v3 (SPMD over 8 NeuronCores).

Layout: partitions = j (each core owns 256 j's as 2 tiles of 128), free
dim = i. Each tile gets its OWN permutation of the i axis: columns
sorted by min-distance to the tile's 128 atoms (atoms are k-d ordered so
a 128-block's neighborhood is compact), so the 512-column prefix holds
every pair within the vdW cutoff. Electrostatics run full width; the
vdW chain (sigmoids + 3 Gaussians via Derivative_Erf + repulsion exp)
runs on shrinking prefixes (448/384/320/256).

D^2 comes from one k=13 fp16 hi/lo-split Gram matmul (PE multiplies
fp16 exactly, PSUM accumulates fp32; |err| ~ 5e-4). Near pairs
(D^2 < 0.25) and the diagonal get +1e6 pokes so their device
contribution is ~0 (elec) / exactly 0 (vdW); the host adds their exact
fp64 contributions.

Both reduction stages run on the PE: stage 1 uses the maps (invD,
invD2, vdw) as stationary lhsT against per-j weight columns, giving
per-i partials; stage 2 contracts those over i with per-atom weight
columns (qs, qs*c, qs*sfb, qs*sfb*c, 1, c), accumulating 6x6 / 2x6
energy cells in PSUM. The host combines 2*16*8 fp32 cells in fp64.
"""
import numpy as np
import ml_dtypes

import concourse.bass as bass
import concourse.mybir as mybir
import bass_rust as _bass_rust
from concourse.bass_utils import run_bass_kernel_spmd
from concourse.tile import TileContext

N = 2048
C = 8
CONV = 332.07156
NCORES = 8
P = 128
JT = 2          # j-tiles per core
WC = 512        # compact prefix (pokes + vdW support)
WEA = 448       # width for exp(-(Dm-0.3)^2) and the vdw map
WE3 = 384       # width for exp(-3 Dm^2)
WE10 = 384      # width for exp(-10 Dm^2); 3 full 128-blocks
WRP = 256       # width for repulsion 5 exp(-0.3 D^3)
CUT = 9.0       # neighbor cutoff (A) for the compact prefix
NEAR_TH2 = 0.25
POKE = 1.0e6
NB = 16         # stage-1 i-blocks of 128
LN5 = float(np.log(5.0))
SQ3 = float(np.sqrt(3.0))
SQ10 = float(np.sqrt(10.0))
SQPI = float(np.sqrt(np.pi))

AF = mybir.ActivationFunctionType
ALU = mybir.AluOpType
F32 = mybir.dt.float32
BF16 = mybir.dt.bfloat16
FP16 = mybir.dt.float16


# --------------------------------------------------------------- patches
def _patched_drain_and_barrier(self, tick_clock, wait_clock):
    gc = tick_clock.global_clock
    try:
        n_procs = len(gc)
    except TypeError:
        n_procs = 27
    ticks = [gc[p] for p in range(n_procs)]
    for p in [p for p in range(n_procs) if ticks[p] > 0] or [0]:
        d = self.nc.sync.drain()
        sub = [ticks[q] if q == p else 0 for q in range(n_procs)]
        wait_clock.add_sem_waits(
            d.ins, _bass_rust.ScopedClock({None: _bass_rust.VectorClock(sub)})
        )
    self.nc.all_engine_barrier()
    assert self.sems is not None
    popped = self.nc._tile_sem_poison_stack.pop()
    assert popped is self._sem_poison
    self.nc.clear_and_free_semaphores(list(self.sems.allocated().values()))


TileContext._drain_and_barrier = _patched_drain_and_barrier

_NOPC = [0]


def _split_excess_waits(nc):
    """This walrus build rejects instructions carrying more than one sem
    wait. Hoist excess waits onto same-engine NoOps inserted just before
    the offending instruction (the engine sequencer executes them in
    order, so the waits still gate it)."""
    for blk in nc.m.functions[0].blocks:
        insts = blk.instructions
        out = []
        changed = False
        for inst in insts:
            si = inst.sync_info
            waits = list(si.on_wait) if si is not None else []
            if len(waits) > 1:
                keep_idx = len(waits) - 1
                if type(inst).__name__ == "InstDMACopy":
                    for k, w in enumerate(waits):
                        if str(getattr(w, "ant_name", "")).startswith(
                                ("DMAHW", "DMASW")):
                            keep_idx = k
                            break
                rest = [w for k, w in enumerate(waits) if k != keep_idx]
                for w in rest:
                    _NOPC[0] += 1
                    nop = mybir.InstNoOp(name=f"WH-{_NOPC[0]}", ins=[], outs=[])
                    nop.engine = inst.engine
                    nop.sync_info = mybir.SyncInfo(on_wait=[w], on_update=[])
                    out.append(nop)
                inst.sync_info = mybir.SyncInfo(on_wait=[waits[keep_idx]],
                                                on_update=list(si.on_update))
                changed = True
            out.append(inst)
        if changed:
            blk.instructions = out


def _bcast_src(dram_ap, n_free):
    """Stride-0 partition AP: read one DRAM row into all 128 partitions."""
    return bass.AP(tensor=dram_ap.tensor, offset=0,
                   ap=_bass_rust.VecI64Pair([[0, P], [1, n_free]]))


_CACHE = {}


def _build():
    if "nc" in _CACHE:
        return _CACHE["nc"]
    nc = bass.Bass()
    # geo cols: [0:N rhs | N:N+P lhsT | +P one-hot lhsT | +2*WEA br/bd packs]
    GEOW = N + P + P + 2 * WEA
    geo = [nc.declare_dram_parameter(f"geo{t}", [13, GEOW], FP16,
                                     isOutput=False) for t in range(JT)]
    WPK = 256   # poke window (all poked columns sort below count(minD<.5))
    pks = nc.declare_dram_parameter("pks", [P, P + 2 * WPK], BF16,
                                    isOutput=False)
    scal_d = nc.declare_dram_parameter("scal", [P, 8 * JT], F32,
                                       isOutput=False)
    scw = nc.declare_dram_parameter("scw", [P, 6 * NB * JT], F32,
                                    isOutput=False)
    wts = nc.declare_dram_parameter("wts", [P, 8 * JT], FP16, isOutput=False)
    cells_out = nc.declare_dram_parameter("cells", [66, 18], F32,
                                          isOutput=True)

    with TileContext(nc) as tc:
        with tc.tile_pool(name="const", bufs=1) as cpool, \
             tc.tile_pool(name="work", bufs=1) as wpool, \
             tc.tile_pool(name="gpin", bufs=2, space="PSUM") as gpin, \
             tc.tile_pool(name="gring", bufs=2, space="PSUM") as gring, \
             tc.tile_pool(name="pvals", bufs=1, space="PSUM") as pvals, \
             tc.tile_pool(name="pbc", bufs=1, space="PSUM") as pbc, \
             tc.tile_pool(name="pout", bufs=1, space="PSUM") as poutp:

            t_geofull = [cpool.tile([13, N + P + P + 2 * WEA], FP16,
                                    name=f"t_geo{t}") for t in range(JT)]
            t_geo = [g[:, 0:N] for g in t_geofull]
            t_geoT = [g[:, N:N + P] for g in t_geofull]
            t_bone = [g[:, N + P:N + 2 * P] for g in t_geofull]
            t_bpack = [g[:, N + 2 * P:] for g in t_geofull]
            t_pks = cpool.tile([P, P + 2 * 256], BF16, name="t_pks")
            t_pkid = t_pks[:, 0:P]
            t_pk = [t_pks[:, P + t * 256:P + (t + 1) * 256]
                    for t in range(JT)]
            t_scal_t = cpool.tile([P, 8 * JT], F32, name="t_scal")
            t_scal = t_scal_t[:, :]
            t_scw = cpool.tile([P, 6 * NB * JT], F32, name="t_scw")
            t_wq = t_scw[:, :]
            t_wts = cpool.tile([P, 8 * JT], FP16, name="t_wts")
            # DMA order = need order
            nc.sync.dma_start(t_geofull[0][:], geo[0][:])
            nc.sync.dma_start(t_scal_t[:], scal_d[:])
            nc.sync.dma_start(t_pks[:], pks[:])
            nc.sync.dma_start(t_geofull[1][:], geo[1][:])
            nc.sync.dma_start(t_scw[:], scw[:])
            nc.sync.dma_start(t_wts[:], wts[:])

            # out cells (matmul col base must be 0/32/64):
            # elec t@[32t:32t+6, 0:6], vdw t@[64:66, 6+6t:12+6t]
            t_out = poutp.tile([66, 18], F32, name="t_out")
            t_out_e = t_out[:, 0:6]
            t_out_v = t_out[:, 0:18]

            def sc(t, k):
                return t_scal[:, 8 * t + k:8 * t + k + 1]

            # ---- sigmoid args via k=3 PE matmul: br_i/bd_i data row plus
            # ar_j/ad_j bias rows against segment indicators; one bias-free
            # sigmoid over both segments reads the PSUM directly
            sigr, sigd = [], []
            for t in range(JT):
                pb = pbc.tile([P, 2 * WEA], F32, name=f"bc_{t}", tag="bc")
                nc.tensor.matmul(pb[:, 0:WEA], t_bone[t][0:3, :],
                                 t_bpack[t][0:3, 0:WEA],
                                 start=True, stop=True)
                nc.tensor.matmul(pb[:, WEA:2 * WEA], t_bone[t][0:3, :],
                                 t_bpack[t][0:3, WEA:2 * WEA],
                                 start=True, stop=True)
                sg = wpool.tile([P, 2 * WEA], FP16, name=f"sigs_{t}")
                nc.scalar.activation(sg[:, 0:WEA], pb[:, 0:WEA], AF.Sigmoid)
                nc.scalar.activation(sg[:, WEA:2 * WEA], pb[:, WEA:2 * WEA],
                                     AF.Sigmoid)
                sigr.append(sg[:, 0:WEA])
                sigd.append(sg[:, WEA:2 * WEA])

            # ---- per tile: Gram -> invD2 -> invD -> D_c -> Dm
            invD, invD2, D_c, Dm = [], [], [], []
            for t in range(JT):
                Gb = []
                for cidx in range(4):
                    if cidx == 0:
                        g = gpin.tile([P, 512], F32, name=f"G_{t}_0",
                                      tag="Gpin")
                        nc.tensor.matmul(g[:, 0:256], t_geoT[t][:],
                                         t_geo[t][0:13, 0:256],
                                         start=True, stop=False)
                        nc.tensor.matmul(g[:, 0:256], t_pkid[:], t_pk[t][:],
                                         start=False, stop=True)
                        nc.tensor.matmul(g[:, 256:512], t_geoT[t][:],
                                         t_geo[t][0:13, 256:512],
                                         start=True, stop=True)
                    else:
                        g = gring.tile([P, 512], F32, name=f"G_{t}_{cidx}",
                                       tag="G")
                        cs = 512 * cidx
                        nc.tensor.matmul(g[:], t_geoT[t][:],
                                         t_geo[t][0:13, cs:cs + 512],
                                         start=True, stop=True)
                    Gb.append(g)

                iD2 = wpool.tile([P, N], FP16, name=f"invD2_{t}")
                with nc.allow_low_precision(reason="fp16 maps; reductions "
                                            "accumulate fp32 in PSUM"):
                    for cidx in range(4):
                        sl = slice(cidx * 512, (cidx + 1) * 512)
                        nc.vector.reciprocal(iD2[:, sl], Gb[cidx][:])
                iD = wpool.tile([P, N], FP16, name=f"invD_{t}")
                for h in range(2):
                    sl = slice(h * 1024, (h + 1) * 1024)
                    nc.scalar.activation(iD[:, sl], iD2[:, sl], AF.Sqrt)
                invD2.append(iD2)
                invD.append(iD)

                # D_c needs only invD[:, 0:WEA] (first sqrt half) + G0
                dc = wpool.tile([P, WEA], FP16, name=f"Dc_{t}")
                nc.vector.tensor_tensor(dc[:], Gb[0][:, 0:WEA],
                                        iD[:, 0:WEA], ALU.mult)
                s_m = wpool.tile([P, WEA], FP16, name=f"s_{t}")
                nc.vector.tensor_scalar(s_m[:], sigr[t][:], sc(t, 2),
                                        sc(t, 3), ALU.mult, ALU.add)
                dm = wpool.tile([P, WEA], FP16, name=f"Dm_{t}")
                nc.vector.tensor_tensor(dm[:], dc[:], s_m[:], ALU.subtract)
                D_c.append(dc)
                Dm.append(dm)

            # ---- per tile: Gaussians + repulsion + vdw map + reductions
            for t in range(JT):
                ea = wpool.tile([P, WEA], BF16, name=f"ea_{t}")
                nc.scalar.activation(ea[:], Dm[t][:], AF.Derivative_Erf,
                                     bias=sc(t, 6))
                eb = wpool.tile([P, WE3], BF16, name=f"eb_{t}")
                nc.scalar.activation(eb[:], Dm[t][:, 0:WE3],
                                     AF.Derivative_Erf, scale=SQ3)
                ec = wpool.tile([P, WE10], BF16, name=f"ec_{t}")
                nc.scalar.activation(ec[:], Dm[t][:, 0:WE10],
                                     AF.Derivative_Erf, scale=SQ10)
                w3 = wpool.tile([P, WEA], BF16, name=f"w3_{t}")
                nc.vector.tensor_scalar(w3[:], sigd[t][:], sc(t, 4), sc(t, 5),
                                        ALU.mult, ALU.add)
                WS = wpool.tile([P, WC], BF16, name=f"WS_{t}")
                nc.gpsimd.memset(WS[:, WEA:WC], 0.0)
                nc.vector.tensor_tensor(WS[:, 0:WEA], w3[:], ea[:], ALU.mult)
                WB = wpool.tile([P, WE3], BF16, name=f"WB_{t}")
                nc.vector.tensor_tensor(WB[:], w3[:, 0:WE3], eb[:], ALU.mult)
                WE = wpool.tile([P, WE10], BF16, name=f"WE_{t}")
                nc.vector.tensor_tensor(WE[:], w3[:, 0:WE10], ec[:], ALU.mult)

                D2c = wpool.tile([P, WRP], BF16, name=f"D2c_{t}")
                nc.gpsimd.tensor_tensor(D2c[:], D_c[t][:, 0:WRP],
                                        D_c[t][:, 0:WRP], ALU.mult)
                D3 = wpool.tile([P, WRP], BF16, name=f"D3_{t}")
                nc.gpsimd.tensor_tensor(D3[:], D2c[:], D_c[t][:, 0:WRP],
                                        ALU.mult)
                repl = wpool.tile([P, WRP], BF16, name=f"repl_{t}")
                nc.scalar.activation(repl[:], D3[:], AF.Exp, scale=-0.3,
                                     bias=sc(t, 7))

                # ---- stage 1: per-i partials (maps as stationary lhsT)
                # vals: elec 6/block (0:96), vdw 2/block: WS 4 blocks
                # (96:104), WE 3 blocks (104:110), repl 2 blocks (110:114)
                vals = pvals.tile([P, 120], F32, name=f"vals_{t}",
                                  tag="vals")
                for b in range(NB):
                    bl = slice(b * P, (b + 1) * P)
                    nc.tensor.matmul(vals[:, 6 * b:6 * b + 4],
                                     invD[t][:, bl], t_wts[:, 8 * t:8 * t + 4],
                                     start=True, stop=True)
                    nc.tensor.matmul(vals[:, 6 * b + 4:6 * b + 6],
                                     invD2[t][:, bl],
                                     t_wts[:, 8 * t + 2:8 * t + 4],
                                     start=True, stop=True)
                for b in range(4):
                    nc.tensor.matmul(vals[:, 96 + 2 * b:96 + 2 * b + 2],
                                     WS[:, b * P:(b + 1) * P],
                                     t_wts[:, 8 * t + 4:8 * t + 6],
                                     start=True, stop=True)
                for b in range(3):
                    nc.tensor.matmul(vals[:, 104 + 2 * b:104 + 2 * b + 2],
                                     WB[:, b * P:(b + 1) * P],
                                     t_wts[:, 8 * t + 4:8 * t + 6],
                                     start=True, stop=True)
                for b in range(3):
                    nc.tensor.matmul(vals[:, 110 + 2 * b:110 + 2 * b + 2],
                                     WE[:, b * P:(b + 1) * P],
                                     t_wts[:, 8 * t + 4:8 * t + 6],
                                     start=True, stop=True)
                for b in range(2):
                    nc.tensor.matmul(vals[:, 116 + 2 * b:116 + 2 * b + 2],
                                     repl[:, b * P:(b + 1) * P],
                                     t_wts[:, 8 * t + 6:8 * t + 8],
                                     start=True, stop=True)

                svals = wpool.tile([P, 120], F32, name=f"sv_{t}")
                nc.vector.tensor_scalar(svals[:, 0:96], vals[:, 0:96],
                                        1.0, None, ALU.mult)
                nc.vector.tensor_scalar(svals[:, 96:120], vals[:, 96:120],
                                        1.0, None, ALU.mult)

                # ---- stage 2: contract over i with per-atom weights
                for b in range(NB):
                    nc.tensor.matmul(t_out_e[32 * t:32 * t + 6, 0:6],
                                     svals[:, 6 * b:6 * b + 6],
                                     t_wq[:, 6 * NB * t + 6 * b:
                                          6 * NB * t + 6 * b + 6],
                                     start=(b == 0), stop=(b == NB - 1))
                v2 = ([(96 + 2 * b, b) for b in range(4)]
                      + [(104 + 2 * b, b) for b in range(3)]
                      + [(110 + 2 * b, b) for b in range(3)]
                      + [(116 + 2 * b, b) for b in range(2)])
                for k, (col, b) in enumerate(v2):
                    nc.tensor.matmul(t_out_v[64:66, 6 + 6 * t:12 + 6 * t],
                                     svals[:, col:col + 2],
                                     t_wq[:, 6 * NB * t + 6 * b:
                                          6 * NB * t + 6 * b + 6],
                                     start=(k == 0), stop=(k == len(v2) - 1))

            # ---- evacuate the cell groups (partition-aligned copies into
            # one zeroed SBUF tile, then a single DMA)
            sb_out = wpool.tile([66, 18], F32, name="sb_out")
            nc.gpsimd.memset(sb_out[:], 0.0)
            for t in range(JT):
                nc.vector.tensor_scalar(sb_out[32 * t:32 * t + 6, 0:6],
                                        t_out[32 * t:32 * t + 6, 0:6],
                                        1.0, None, ALU.mult)
            nc.vector.tensor_scalar(sb_out[64:66, :],
                                    t_out[64:66, :],
                                    1.0, None, ALU.mult)
            nc.sync.dma_start(cells_out[:], sb_out[:])

    _split_excess_waits(nc)
    _CACHE["nc"] = nc
    return nc


# --------------------------------------------------------------- host side
def _kd_order(X):
    out = []

    def rec(ids):
        if len(ids) <= P:
            out.append(ids)
            return
        spans = X[ids].max(0) - X[ids].min(0)
        ax = int(np.argmax(spans))
        order = ids[np.argsort(X[ids, ax], kind="stable")]
        half = (len(ids) // 2 // P) * P
        rec(order[:half])
        rec(order[half:])

    rec(np.arange(len(X)))
    return np.concatenate(out)


def _f16_split(x):
    h = x.astype(np.float16)
    l = (x - h.astype(np.float64)).astype(np.float16)
    return h, l


def _host_pre(inputs):
    f32, f64 = np.float32, np.float64
    X = np.asarray(inputs["X"], f32)
    embs = np.asarray(inputs["embs"], f32)
    qs = np.asarray(inputs["qs"], f32)
    w0 = np.asarray(inputs["w0"], f32)
    s0 = np.asarray(inputs["s0"], f32)
    c = np.asarray(inputs["chainidx"]).astype(f32)
    f = np.asarray(inputs["sf_elec"], f32)[:, 0]
    rf = np.asarray(inputs["radius_factor"], f32)[:, 0]
    df = np.asarray(inputs["depth_factor"], f32)[:, 0]

    X64 = X.astype(f64)
    Xc64 = X64 - X64.mean(0)
    r2_64 = (Xc64 ** 2).sum(1)
    D2x = r2_64[:, None] + r2_64[None, :] - 2.0 * (Xc64 @ Xc64.T)
    np.fill_diagonal(D2x, 0.0)
    D2x = np.maximum(D2x, 0.0)

    perm = _kd_order(Xc64)

    # sorted-frame quantities
    r2s = r2_64[perm]
    D2s = D2x[perm][:, perm]
    Xs = Xc64[perm]
    sfa = (embs @ f[:C]).astype(f64)[perm]
    sfb = (embs @ f[C:2 * C]).astype(f64)[perm]
    f16 = float(f[2 * C])
    ar = (embs @ rf[:C]).astype(f64)[perm]
    br = (embs @ rf[C:]).astype(f64)[perm]
    ad = (embs @ df[:C]).astype(f64)[perm]
    bd = (embs @ df[C:]).astype(f64)[perm]
    w0j = np.sqrt(w0.astype(f64) ** 2 + 1e-6)[perm]
    qs_s = qs.astype(f64)[perm]
    c_s = c.astype(f64)[perm]
    s0_s = s0.astype(f64)[perm]

    hx, lx = _f16_split(Xs)
    hr2j, lr2j = _f16_split(r2s)
    hr2i, lr2i = _f16_split(r2s + 3e-6)

    pkid_m = (np.eye(P, dtype=f32) * POKE).astype(ml_dtypes.bfloat16)
    u3 = qs_s * c_s
    u4 = qs_s * (1.0 - 2.0 * c_s)

    in_maps = []
    for core in range(NCORES):
        m = {}
        pks_m = np.zeros((P, P + 2 * 256), f32)
        pks_m[:, 0:P] = np.eye(P, dtype=f32) * POKE
        scal_m = np.zeros((P, 8 * JT), f32)
        wts_m = np.zeros((P, 8 * JT), f32)
        wq_m = np.zeros((P, 6 * NB * JT), f32)
        for t in range(JT):
            g0 = core * (P * JT) + t * P
            jj = slice(g0, g0 + P)
            minD2 = D2s[jj].min(0)
            pi = np.argsort(minD2, kind="stable")   # full 2048 permutation

            geo_m = np.zeros((13, N + P + P + 2 * WEA), np.float16)
            geo_r = geo_m[:, 0:N]
            geoT_m = geo_m[:, N:N + P]
            geo_m[0, N + P:N + 2 * P] = 1.0
            geo_m[1, N + P:N + 2 * P] = ar[jj].astype(np.float16)
            geo_m[2, N + P:N + 2 * P] = ad[jj].astype(np.float16)
            geo_m[0, N + 2 * P:N + 2 * P + WEA] = br[pi[:WEA]].astype(
                np.float16)
            geo_m[0, N + 2 * P + WEA:] = bd[pi[:WEA]].astype(np.float16)
            geo_m[1, N + 2 * P:N + 2 * P + WEA] = 1.0
            geo_m[2, N + 2 * P + WEA:] = 1.0
            for d in range(3):
                geo_r[3 * d + 0] = hx[pi, d]
                geo_r[3 * d + 1] = lx[pi, d]
                geo_r[3 * d + 2] = hx[pi, d]
                geoT_m[3 * d + 0] = -2.0 * hx[jj, d]
                geoT_m[3 * d + 1] = -2.0 * hx[jj, d]
                geoT_m[3 * d + 2] = -2.0 * lx[jj, d]
            geo_r[9] = 1.0
            geo_r[10] = 1.0
            geoT_m[9] = hr2j[jj]
            geoT_m[10] = lr2j[jj]
            geo_r[11] = hr2i[pi]
            geo_r[12] = lr2i[pi]
            geoT_m[11] = 1.0
            geoT_m[12] = 1.0

            pos = np.empty(N, np.int64)
            pos[pi] = np.arange(N)
            pk_m = np.zeros((P, 256), f32)
            assert pos[g0 + np.arange(P)].max() < 256
            pk_m[np.arange(P), pos[g0 + np.arange(P)]] = 1.0
            nj, ni_ = np.where(D2s[jj] < NEAR_TH2)
            sel = ni_ != (g0 + nj)
            assert len(ni_) == 0 or pos[ni_[sel]].max() < 256
            pk_m[nj[sel], pos[ni_[sel]]] = 1.0

            m[f"geo{t}"] = geo_m
            pks_m[:, P + t * 256:P + (t + 1) * 256] = pk_m

            scal_m[:, 8 * t + 0] = ar[jj]
            scal_m[:, 8 * t + 1] = ad[jj]
            scal_m[:, 8 * t + 2] = 1.6 * s0_s[jj]
            scal_m[:, 8 * t + 3] = 0.8 * s0_s[jj]
            scal_m[:, 8 * t + 4] = w0j[jj] * (SQPI / 6.0)
            scal_m[:, 8 * t + 5] = w0j[jj] * (SQPI / 12.0)
            scal_m[:, 8 * t + 6] = -0.3
            scal_m[:, 8 * t + 7] = LN5
            wts_m[:, 8 * t + 0] = u3[jj] * sfa[jj]
            wts_m[:, 8 * t + 1] = u4[jj] * sfa[jj]
            wts_m[:, 8 * t + 2] = u3[jj]
            wts_m[:, 8 * t + 3] = u4[jj]
            wts_m[:, 8 * t + 4] = c_s[jj]
            wts_m[:, 8 * t + 5] = 1.0 - 2.0 * c_s[jj]
            wts_m[:, 8 * t + 6] = -c_s[jj]
            wts_m[:, 8 * t + 7] = -(1.0 - 2.0 * c_s[jj])
            for b in range(NB):
                ib = pi[b * P:(b + 1) * P]
                base = 6 * NB * t + 6 * b
                wq_m[:, base + 0] = qs_s[ib]
                wq_m[:, base + 1] = qs_s[ib] * c_s[ib]
                wq_m[:, base + 2] = qs_s[ib] * sfb[ib]
                wq_m[:, base + 3] = qs_s[ib] * sfb[ib] * c_s[ib]
                wq_m[:, base + 4] = 1.0
                wq_m[:, base + 5] = c_s[ib]
        m["pks"] = pks_m.astype(ml_dtypes.bfloat16)
        m["scal"] = scal_m
        m["scw"] = wq_m
        m["wts"] = wts_m.astype(np.float16)
        in_maps.append(m)

    # ---- exact fp64 contributions of the poked near pairs (device ~0)
    ni_a, nj_a = np.where((D2s < NEAR_TH2) & (D2s > 0))
    e_elec_corr = 0.0
    e_vdw_corr = 0.0
    if len(ni_a):
        msk = c_s[ni_a] != c_s[nj_a]
        ia, ja = ni_a[msk], nj_a[msk]       # ordered pairs, both directions
        Dn = np.sqrt(D2s[ia, ja] + 3e-6)
        invDn = 1.0 / (Dn + 1e-6)
        e_elec_corr = 0.5 * CONV * np.sum(
            qs_s[ia] * qs_s[ja] * invDn
            * (sfa[ja] + sfb[ia] + f16 * invDn))
        sig_r = 1.0 / (1.0 + np.exp(-(ar[ja] + br[ia])))
        s = 2.0 * s0_s[ja] * (0.8 * sig_r + 0.4)
        repl = 5.0 * np.exp(-0.3 * Dn ** 3)
        Dmn = Dn - s
        attr = (np.exp(-(Dmn - 0.3) ** 2) + np.exp(-3.0 * Dmn * Dmn)
                + np.exp(-10.0 * Dmn * Dmn)) / 3.0
        sig_d = 1.0 / (1.0 + np.exp(-(ad[ja] + bd[ia])))
        w = w0j[ja] * (sig_d + 0.5)
        e_vdw_corr = np.sum(-w * attr + repl)

    aux = dict(inputs=inputs, f16=f16,
               e_elec_corr=e_elec_corr, e_vdw_corr=e_vdw_corr)
    return in_maps, aux


def _host_post(core_cells, aux):
    f64 = np.float64
    f16 = aux["f16"]
    E_elec = 0.0
    E_vdw = 0.0
    for cells in core_cells:
        cc = cells.astype(f64)
        for t in range(JT):
            e = cc[32 * t:32 * t + 6, 0:6]
            v = cc[64:66, 6 + 6 * t:12 + 6 * t]
            E_elec += (e[0, 0] + e[1, 1] + e[2, 2] + e[3, 3]
                       + f16 * (e[4, 0] + e[5, 1]))
            E_vdw += -(v[0, 4] + v[1, 5])
    E_elec = 0.5 * CONV * E_elec + aux["e_elec_corr"]
    E_vdw = E_vdw + aux["e_vdw_corr"]

    inputs = aux["inputs"]
    embs = np.asarray(inputs["embs"], np.float32)
    die = np.asarray(inputs["die_factor"], np.float32)
    born = np.asarray(inputs["born_factor"], np.float32)
    qsf = np.asarray(inputs["qs"], np.float32).astype(f64)
    atomic_die = (embs @ die + 1e-6).astype(f64)
    R = (embs @ born + 1.0).astype(f64)
    E_self = -(1.0 - 1.0 / atomic_die) * qsf / (R + 1e-6)
    E_solv = CONV * np.sum(E_self) * 0.01

    def guard(e):
        return np.float32(1e-6) if np.isnan(e) else np.float32(e)

    return np.asarray([guard(E_vdw), guard(E_elec), guard(E_solv)],
                      dtype=np.float32)


def kernel(**inputs):
    nc = _build()
    in_maps, aux = _host_pre(inputs)
    res = run_bass_kernel_spmd(nc, in_maps, list(range(NCORES)))
    core_cells = [res.results[cid]["cells"] for cid in range(NCORES)]
    return _host_post(core_cells, aux)


if __name__ == "__main__":
    pass
